# revision 32
# baseline (speedup 1.0000x reference)
"""AttentionBlock3D (GroupNorm + single-head attention over 4096 tokens + residual)
on 8 Trainium2 NeuronCores.

Sharding: core c handles sample b=c//2 and query half h=c%2 (2048 queries).
The host rotates each sample's token axis so that each core's queries are
columns 0..2048 of its x_kv input (attention is permutation-invariant over
keys, and groupnorm stats are permutation-invariant), letting a single SPMD
program serve all 8 cores with no dynamic offsets.

Device-side math per core:
  h = groupnorm(x)                   fp32 stats (bn_stats + indicator-matmul
                                     cross-partition group reduce), fp16 output
  q = Wq h + bq  [256,2048] fp16     (host passes qkv_w^T; no on-chip transposes)
  k = Wk h + bk  [256,4096] fp16
  vT = (Wv h)^T  [4096,256] fp16     (computed directly in transposed layout)
  S^T tiles [128 keys, 512 queries] -> es = exp(S^T * C^-0.5) fp16
                                     (no max subtraction: |scores*scale| <~ 8)
  pv[c,q]  = sum_m vT[m,c] es[m,q]   (v chunks stationary, es streams, fp32 psum)
  den[q]   = sum_m es[m,q]           (ones-column matmul)
  fin      = out_w @ pv              (normalization deferred: a per-query column
                                      scale commutes through the projection)
  y = fin * (1/den bcast) + (out_w @ bv + ob) + x   (v-bias folded via sum(A)=1)

All matmuls run in fp16 (fp32 on the PE lowers to 2 HW passes at half rate);
PSUM accumulation is fp32 throughout, softmax statistics are fp32.
"""
import numpy as np

_CACHE: dict = {}

B, C, N = 4, 256, 4096
NQ = 2048          # queries per core
G = 8              # groups
GS = 32            # channels per group
EPS = 1e-5
SCALE = C ** -0.5


def _install_tile_drain_patch():
    """This container's walrus only supports ONE sync-wait command per
    instruction; TileContext's final drain carries one wait per live proc and
    fails codegen. Re-emit the waits as standalone sem-wait instructions."""
    import concourse.tile as tile_mod
    from concourse.vector_clock import ScopedClock

    if getattr(tile_mod.TileContext, "_ant_drain_patched", False):
        return

    def _patched(self, tick_clock, wait_clock):
        nc = self.nc
        probe = nc.sync.nop(nofuse=True)
        wait_clock.add_sem_waits(
            probe.ins, ScopedClock({None: tick_clock.global_clock})
        )
        waits = list(probe.ins.sync_info.on_wait or [])
        if len(waits) > 1:
            probe.ins.sync_info.on_wait = []
            num2h = {}
            assert self.sems is not None
            for h in self.sems.allocated().values():
                num2h[h.num] = h
            for h in self.sems.swdge_block():
                num2h.setdefault(h.num, h)
            for w in waits:
                h = num2h.get(w.id)
                assert h is not None, f"sem id {w.id} ({w.ant_name}) has no handle"
                nc.sync.wait_op(h, w.wait_value, "sem-ge")
        nc.sync.drain()
        nc.all_engine_barrier()
        assert self.sems is not None
        popped = nc._tile_sem_poison_stack.pop()
        assert popped is self._sem_poison
        nc.clear_and_free_semaphores(list(self.sems.allocated().values()))

    tile_mod.TileContext._drain_and_barrier = _patched
    tile_mod.TileContext._ant_drain_patched = True


def _split_multiwait_instructions(nc):
    """This walrus build supports only one sync-wait command per instruction.
    Hoist extra waits into standalone InstEventSemaphore waits on the same
    engine immediately before the instruction (sequential single waits are
    equivalent to one multi-wait)."""
    import concourse.mybir as mybir

    n = 0
    for f in nc.m.functions:
        for bb in f.blocks:
            new = []
            for ins in bb.instructions:
                si = ins.sync_info
                waits = list(si.on_wait) if si else []
                if len(waits) > 1:
                    for w in waits[:-1]:
                        nop = mybir.InstEventSemaphore(
                            name=f"WSPLIT-{n}", ins=[], outs=[])
                        n += 1
                        nop.engine = ins.engine
                        nop.sync_info = mybir.SyncInfo(on_wait=[w], on_update=[])
                        new.append(nop)
                    si.on_wait = [waits[-1]]
                new.append(ins)
            bb.instructions[:] = new


def _build():
    import concourse.bass as bass
    import concourse.tile as tile
    import concourse.mybir as mybir
    from contextlib import ExitStack

    _install_tile_drain_patch()

    f32 = mybir.dt.float32
    f16 = mybir.dt.float16
    Act = mybir.ActivationFunctionType
    Alu = mybir.AluOpType

    nc = bass.Bass()
    x_kv = nc.dram_tensor("x_kv", [C, N], f32, kind="ExternalInput")
    # w_pack = [qkv_wT | out_wT] : [256, 1024]; vec_pack cols =
    # qb0..qb5, nw0, nw1, nb0, nb1, ob0, ob1 : [128, 12]
    w_pack = nc.dram_tensor("w_pack", [C, 4 * C], f32, kind="ExternalInput")
    vec_pack = nc.dram_tensor("vec_pack", [128, 12], f32, kind="ExternalInput")
    y = nc.dram_tensor("y", [C, NQ], f32, kind="ExternalOutput")

    with ExitStack() as ctx:
        tc = ctx.enter_context(tile.TileContext(nc))
        const = ctx.enter_context(tc.tile_pool(name="const", bufs=1))
        xqp = ctx.enter_context(tc.tile_pool(name="xqp", bufs=16))
        hp = ctx.enter_context(tc.tile_pool(name="hp", bufs=16))
        kp = ctx.enter_context(tc.tile_pool(name="kp", bufs=2))
        qp = ctx.enter_context(tc.tile_pool(name="qp", bufs=2))
        vtp = ctx.enter_context(tc.tile_pool(name="vtp", bufs=32))
        esp = ctx.enter_context(tc.tile_pool(name="esp", bufs=16))
        smal = ctx.enter_context(tc.tile_pool(name="smal", bufs=2))
        sb2 = ctx.enter_context(tc.tile_pool(name="sb2", bufs=2))
        psp = ctx.enter_context(tc.tile_pool(name="psp", bufs=3, space="PSUM"))
        pvp = ctx.enter_context(tc.tile_pool(name="pvp", bufs=2, space="PSUM"))
        denp = ctx.enter_context(tc.tile_pool(name="denp", bufs=1, space="PSUM"))
        finp = ctx.enter_context(tc.tile_pool(name="finp", bufs=2, space="PSUM"))

        # ---- load x FIRST in [128,512] units round-robin over the three
        #      DMA-capable engines (one queue each); finer units mean the
        #      per-unit bn_stats pipelines tightly with arrivals ----
        xq = [[None] * 8 for _ in range(2)]
        dma_engs = [nc.sync, nc.gpsimd, nc.scalar]
        for u in range(8):
            for ct in range(2):
                t = xqp.tile([128, 512], f32, tag="xq", name=f"xq{ct}_{u}")
                eng = dma_engs[(u * 2 + ct) % 3]
                eng.dma_start(
                    out=t,
                    in_=x_kv[ct * 128:(ct + 1) * 128, u * 512:(u + 1) * 512])
                xq[ct][u] = t

        # ---- constants (after the x DMA issues: the tiny ind_bwd
        #      SBUF-to-SBUF DMAs would otherwise delay x on the sync queue) ----
        ones_col = const.tile([128, 128], f16, name="ones_col")
        nc.vector.memset(ones_col, 1.0)
        ones_row = const.tile([1, 128], f32, name="ones_row")
        nc.vector.memset(ones_row, 1.0)
        ind_fwd = const.tile([128, 4], f32, name="ind_fwd")   # [ch, grp] one-hot
        nc.vector.memset(ind_fwd, 0.0)
        for g in range(4):
            nc.vector.memset(ind_fwd[g * GS:(g + 1) * GS, g:g + 1], 1.0)
        ind_bwd = const.tile([4, 128], f32, name="ind_bwd")   # [grp, ch] one-hot
        nc.vector.memset(ind_bwd[0:4, :], 0.0)
        for g in range(4):
            # engines can't start at partition g∉{0,32,64,96}; DMA can
            nc.sync.dma_start(out=ind_bwd[g:g + 1, g * GS:(g + 1) * GS],
                              in_=ones_row[0:1, 0:GS])
        eps_t = const.tile([128, 1], f32, name="eps_t")
        nc.vector.memset(eps_t, EPS)

        # ---- packed weights / biases (2 + 1 DMA issues) ----
        wpk = []
        for ci in range(2):
            w = const.tile([128, 4 * C], f32, name=f"wpk{ci}")
            for hh in range(2):
                dma_engs[(ci * 2 + hh) % 3].dma_start(
                    out=w[:, hh * 512:(hh + 1) * 512],
                    in_=w_pack[ci * 128:(ci + 1) * 128,
                               hh * 512:(hh + 1) * 512])
            wpk.append(w)
        vp = const.tile([128, 12], f32, name="vp")
        nc.gpsimd.dma_start(out=vp, in_=vec_pack[:, :])
        wt = [wpk[ci][:, 0:3 * C] for ci in range(2)]
        owt = [wpk[ci][:, 3 * C:4 * C] for ci in range(2)]
        qb = [vp[:, j:j + 1] for j in range(6)]
        nw = [vp[:, 6 + ci:7 + ci] for ci in range(2)]
        nb = [vp[:, 8 + ci:9 + ci] for ci in range(2)]
        ob = [vp[:, 10 + ci:11 + ci] for ci in range(2)]

        # fp16 casts of matmul weights (fp32 on PE = 2 HW passes at half rate)
        wt16, owt16 = [], []
        for ci in range(2):
            w16 = const.tile([128, 3 * C], f16, name=f"wt16_{ci}")
            nc.vector.tensor_copy(out=w16, in_=wt[ci])
            wt16.append(w16)
            o16 = const.tile([128, C], f16, name=f"owt16_{ci}")
            nc.vector.tensor_copy(out=o16, in_=owt[ci])
            owt16.append(o16)
        qb16v = []
        for j in (4, 5):
            t16 = const.tile([128, 1], f16, name=f"qb16_{j}")
            nc.vector.tensor_copy(out=t16, in_=qb[j])
            qb16v.append(t16)

        # one bn_stats per arriving [128,512] unit (DVE)
        stats = [smal.tile([128, 8, 6], f32, tag=f"stats{ct}", name=f"stats{ct}")
                 for ct in range(2)]
        for u in range(8):
            for ct in range(2):
                nc.vector.bn_stats(out=stats[ct][:, u, :], in_=xq[ct][u])

        # paired layout: mv = [m0, m1, v0, v1] via strided aggr outputs, so
        # every chain op handles both ct tiles at once
        mv = smal.tile([128, 4], f32, tag="mv", name="mv0")
        nc.vector.bn_aggr(out=mv[:, 0:4:2], in_=stats[0])   # cols 0,2
        nc.vector.bn_aggr(out=mv[:, 1:4:2], in_=stats[1])   # cols 1,3
        st2 = smal.tile([128, 4], f32, tag="st2", name="st2")  # [m0,m1,E0,E1]
        nc.vector.tensor_copy(out=st2[:, 0:2], in_=mv[:, 0:2])
        nc.vector.tensor_mul(out=st2[:, 2:4], in0=mv[:, 0:2], in1=mv[:, 0:2])
        nc.vector.tensor_add(out=st2[:, 2:4], in0=st2[:, 2:4], in1=mv[:, 2:4])

        # cross-partition group reduce + broadcast back, both cts at once
        gp = psp.tile([4, 4], f32, tag="ps", name="gp")
        nc.tensor.matmul(gp, ind_fwd, st2, start=True, stop=True)
        gsb = smal.tile([4, 4], f32, tag="gsb", name="gsb")
        nc.scalar.mul(out=gsb, in_=gp, mul=1.0 / GS)
        chp = psp.tile([128, 4], f32, tag="ps", name="chp")
        nc.tensor.matmul(chp, ind_bwd, gsb, start=True, stop=True)
        ch = smal.tile([128, 4], f32, tag="ch", name="ch")   # [mu0,mu1,E20,E21]
        nc.vector.tensor_copy(out=ch, in_=chp)
        var = smal.tile([128, 2], f32, tag="var", name="var")
        nc.vector.tensor_mul(out=var, in0=ch[:, 0:2], in1=ch[:, 0:2])
        nc.vector.tensor_sub(out=var, in0=ch[:, 2:4], in1=var)
        std = smal.tile([128, 2], f32, tag="std", name="std")
        nc.scalar.activation(out=std, in_=var, func=Act.Sqrt, bias=eps_t)
        rstd = smal.tile([128, 2], f32, tag="rstd", name="rstd")
        nc.vector.reciprocal(out=rstd, in_=std)
        Aab = smal.tile([128, 2], f32, tag="Aab", name="Aab")
        nc.vector.tensor_mul(out=Aab, in0=vp[:, 6:8], in1=rstd)
        Bab = smal.tile([128, 2], f32, tag="Bab", name="Bab")
        nc.vector.tensor_mul(out=Bab, in0=ch[:, 0:2], in1=Aab)
        nc.vector.tensor_sub(out=Bab, in0=vp[:, 8:10], in1=Bab)
        AB = [(Aab[:, 0:1], Bab[:, 0:1]), (Aab[:, 1:2], Bab[:, 1:2])]

        # apply affine -> fp16 h tiles per 512 cols (ct0 on DVE, ct1 on
        # ACT, in parallel; fine granularity lets convs start per slice)
        h16 = [[None] * 8 for _ in range(2)]
        for s in range(8):
            for ct in range(2):
                ht = hp.tile([128, 512], f16, tag="h16", name=f"h16_{ct}_{s}")
                xsl = xq[ct][s]
                if ct == 0:
                    nc.vector.tensor_scalar(
                        out=ht, in0=xsl,
                        scalar1=AB[ct][0], scalar2=AB[ct][1],
                        op0=Alu.mult, op1=Alu.add)
                else:
                    nc.scalar.activation(
                        out=ht, in_=xsl, func=Act.Identity,
                        bias=AB[ct][1], scale=AB[ct][0])
                h16[ct][s] = ht

        def hsl(ct, nch):  # [128,512] tile for 512-col chunk nch
            return h16[ct][nch]

        def hmt(ct, mt):   # [128,128] slice of h16 for 128-col tile mt
            return h16[ct][mt // 4][:, (mt % 4) * 128:((mt % 4) + 1) * 128]

        # ---- q conv (queries = first NQ cols) ----
        q_sb = [qp.tile([128, NQ], f16, tag="q", name=f"q{co}") for co in range(2)]
        for co in range(2):
            for nch in range(NQ // 512):
                pq = psp.tile([128, 512], f32, tag="ps", name=f"pq{co}_{nch}")
                nc.tensor.matmul(pq, wt16[0][:, co * 128:(co + 1) * 128],
                                 hsl(0, nch), start=True, stop=False)
                nc.tensor.matmul(pq, wt16[1][:, co * 128:(co + 1) * 128],
                                 hsl(1, nch), start=False, stop=True)
                nc.vector.tensor_scalar_add(
                    out=q_sb[co][:, nch * 512:(nch + 1) * 512],
                    in0=pq, scalar1=qb[co])

        # ---- k conv ----
        k_sb = [kp.tile([128, N], f16, tag="k", name=f"k{co}") for co in range(2)]
        for co in range(2):
            for nch in range(N // 512):
                pk = psp.tile([128, 512], f32, tag="ps", name=f"pk{co}_{nch}")
                nc.tensor.matmul(pk, wt16[0][:, 256 + co * 128:256 + (co + 1) * 128],
                                 hsl(0, nch), start=True, stop=False)
                nc.tensor.matmul(pk, wt16[1][:, 256 + co * 128:256 + (co + 1) * 128],
                                 hsl(1, nch), start=False, stop=True)
                nc.vector.tensor_scalar_add(
                    out=k_sb[co][:, nch * 512:(nch + 1) * 512],
                    in0=pk, scalar1=qb[2 + co])

        # ---- vT conv (no bias; folded into effb) ----
        vt_sb = []
        for mt in range(32):
            pvt = psp.tile([128, 256], f32, tag="ps", name=f"pvt{mt}")
            nc.tensor.matmul(pvt, hmt(0, mt), wt16[0][:, 512:768],
                             start=True, stop=False)
            nc.tensor.matmul(pvt, hmt(1, mt), wt16[1][:, 512:768],
                             start=False, stop=True)
            vt = vtp.tile([128, 256], f16, tag="vt", name=f"vt{mt}")
            nc.vector.tensor_copy(out=vt, in_=pvt)
            vt_sb.append(vt)

        # ---- effective output bias: out_w @ v_bias + out_b ----
        effb = []
        for co in range(2):
            pe_ = psp.tile([128, 1], f32, tag="ps", name=f"peffb{co}")
            nc.tensor.matmul(pe_, owt16[0][:, co * 128:(co + 1) * 128], qb16v[0],
                             start=True, stop=False)
            nc.tensor.matmul(pe_, owt16[1][:, co * 128:(co + 1) * 128], qb16v[1],
                             start=False, stop=True)
            e = smal.tile([128, 1], f32, tag="effb", name=f"effb{co}")
            nc.vector.tensor_add(out=e, in0=pe_, in1=ob[co])
            effb.append(e)

        # ---- attention ----
        def st_chunk(t, mch):
            ps_s = psp.tile([128, 512], f32, tag="ps", name=f"s_{t}_{mch}")
            nc.tensor.matmul(ps_s, k_sb[0][:, mch * 128:(mch + 1) * 128],
                             q_sb[0][:, t * 512:(t + 1) * 512],
                             start=True, stop=False)
            nc.tensor.matmul(ps_s, k_sb[1][:, mch * 128:(mch + 1) * 128],
                             q_sb[1][:, t * 512:(t + 1) * 512],
                             start=False, stop=True)
            es = esp.tile([128, 512], f16, tag="es", name=f"es_{t}_{mch}")
            nc.scalar.activation(out=es, in_=ps_s, func=Act.Exp, scale=SCALE)
            return es

        def epilogue(t, pv, den):
            # den arrives already broadcast across partitions ([128,512]);
            # 1/den is entirely off the PE critical path (normalization
            # commutes through the output projection as a per-query scale).
            # The reciprocal and the y chain run in column halves so the
            # final tile's tail pipelines instead of serializing.
            rdb = sb2.tile([128, 512], f32, tag="rdb", name=f"rdb_{t}")
            for hh in range(2):
                nc.vector.reciprocal(out=rdb[:, hh * 256:(hh + 1) * 256],
                                     in_=den[:, hh * 256:(hh + 1) * 256])
            pvsb = []
            for co in range(2):
                p = sb2.tile([128, 512], f16, tag=f"pvsb{co}",
                             name=f"pvsb_{t}_{co}")
                nc.scalar.copy(out=p, in_=pv[co])  # frees the pv psum bank
                pvsb.append(p)
            fins, rsbs = [], []
            for co in range(2):
                fin = finp.tile([128, 512], f32, tag="fin", name=f"fin_{t}_{co}")
                nc.tensor.matmul(fin, owt16[0][:, co * 128:(co + 1) * 128],
                                 pvsb[0], start=True, stop=False)
                nc.tensor.matmul(fin, owt16[1][:, co * 128:(co + 1) * 128],
                                 pvsb[1], start=False, stop=True)
                rsb = sb2.tile([128, 512], f32, tag=f"rsb{co}", name=f"r_{t}_{co}")
                nc.sync.dma_start(
                    out=rsb,
                    in_=x_kv[co * 128:(co + 1) * 128, t * 512:(t + 1) * 512])
                fins.append(fin)
                rsbs.append(rsb)
            for co in range(2):
                t1 = sb2.tile([128, 512], f32, tag=f"t1{co}", name=f"t1_{t}_{co}")
                ysb = sb2.tile([128, 512], f32, tag=f"ysb{co}", name=f"y_{t}_{co}")
                for hh in range(2):
                    sl = slice(hh * 256, (hh + 1) * 256)
                    nc.vector.tensor_mul(out=t1[:, sl], in0=fins[co][:, sl],
                                         in1=rdb[:, sl])
                    nc.vector.scalar_tensor_tensor(
                        out=ysb[:, sl], in0=t1[:, sl], scalar=effb[co],
                        in1=rsbs[co][:, sl], op0=Alu.add, op1=Alu.add)
                    nc.sync.dma_start(
                        out=y[co * 128:(co + 1) * 128,
                              t * 512 + hh * 256:t * 512 + (hh + 1) * 256],
                        in_=ysb[:, sl])

        NT = NQ // 512
        pend = None
        for t in range(NT):
            # first two S^T chunks (and their exps) are emitted BEFORE the
            # previous tile's epilogue so the ACT stream does exp(t,0/1)
            # first and the PE never waits on the pvsb drains
            es_q = {m: st_chunk(t, m) for m in range(4)}
            if pend is not None:
                epilogue(*pend)   # frees the pv psum banks via pvsb copies
            pv = [pvp.tile([128, 512], f32, tag="pv", name=f"pv_{t}_{co}")
                  for co in range(2)]
            den = denp.tile([128, 512], f32, tag="den", name=f"den_{t}")
            grp, e2s = [], []
            pend_den, den_started = None, False

            def emit_den(rhs, stop=False):
                nonlocal den_started
                nc.tensor.matmul(den, ones_col, rhs,
                                 start=not den_started, stop=stop)
                den_started = True

            for mch in range(32):
                es = es_q.pop(mch)
                st, sp = (mch == 0), (mch == 31)
                nc.tensor.matmul(pv[0], vt_sb[mch][:, 0:128], es, start=st, stop=sp)
                nc.tensor.matmul(pv[1], vt_sb[mch][:, 128:256], es, start=st, stop=sp)
                if mch == 28 and pend_den is not None:
                    emit_den(pend_den)
                    pend_den = None
                if mch >= 28:
                    # tail chunks feed den directly: no GpSimd-add latency at
                    # the tile boundary (den gates the epilogue + next tile)
                    emit_den(es, stop=(mch == 31))
                else:
                    # es quad-sums on the otherwise idle GpSimd engine
                    # quarter the den matmul streams through the PE; each
                    # quad's den matmul is deferred one quad so the PE never
                    # waits on the 3-add GpSimd chain
                    grp.append(es)
                    if len(grp) == 2:
                        e2 = esp.tile([128, 512], f16, tag="es2",
                                      name=f"es2_{t}_{mch}")
                        nc.gpsimd.tensor_add(out=e2, in0=grp[0], in1=grp[1])
                        grp, e2s = [], e2s + [e2]
                        if len(e2s) == 2:
                            q4 = mch // 4
                            e4 = esp.tile([128, 512], f16, tag="es4",
                                          name=f"es4_{t}_{q4}")
                            nc.gpsimd.tensor_add(out=e4, in0=e2s[0], in1=e2s[1])
                            e2s = []
                            if pend_den is not None:
                                emit_den(pend_den)
                            pend_den = e4
                if mch + 4 < 32:
                    es_q[mch + 4] = st_chunk(t, mch + 4)
            pend = (t, pv, den)
        epilogue(*pend)

    _split_multiwait_instructions(nc)
    return nc


def _get_nc():
    if "nc" not in _CACHE:
        _CACHE["nc"] = _build()
    return _CACHE["nc"]


def _prep_in_maps(x, norm_w, norm_b, qkv_w, qkv_b, out_w, out_b):
    xr = np.ascontiguousarray(np.asarray(x, dtype=np.float32).reshape(B, C, N))
    wT = np.asarray(qkv_w, np.float32).T
    owT = np.asarray(out_w, np.float32).T
    w_pack = np.ascontiguousarray(np.concatenate([wT, owT], axis=1))
    qb1 = np.asarray(qkv_b, np.float32)
    vec_pack = np.ascontiguousarray(np.stack(
        [qb1[0:128], qb1[128:256], qb1[256:384], qb1[384:512],
         qb1[512:640], qb1[640:768],
         np.asarray(norm_w, np.float32)[0:128],
         np.asarray(norm_w, np.float32)[128:256],
         np.asarray(norm_b, np.float32)[0:128],
         np.asarray(norm_b, np.float32)[128:256],
         np.asarray(out_b, np.float32)[0:128],
         np.asarray(out_b, np.float32)[128:256]], axis=1))

    in_maps = []
    for c in range(8):
        b, h = divmod(c, 2)
        off = h * NQ
        xb = xr[b]
        if off:
            xroll = np.ascontiguousarray(
                np.concatenate([xb[:, off:], xb[:, :off]], axis=1))
        else:
            xroll = xb
        in_maps.append({"x_kv": xroll, "w_pack": w_pack, "vec_pack": vec_pack})
    return in_maps


def kernel(x, norm_w, norm_b, qkv_w, qkv_b, out_w, out_b):
    from concourse.bass_utils import run_bass_kernel_spmd

    nc = _get_nc()
    in_maps = _prep_in_maps(x, norm_w, norm_b, qkv_w, qkv_b, out_w, out_b)
    res = run_bass_kernel_spmd(nc, in_maps, core_ids=list(range(8)))
    out = np.empty((B, C, N), np.float32)
    for c in range(8):
        b, h = divmod(c, 2)
        off = h * NQ
        out[b][:, off:off + NQ] = res.results[c]["y"]
    return out.reshape(B, C, 16, 16, 16)


# revision 33
# speedup vs baseline: 1.0004x; 1.0004x over previous
"""AttentionBlock3D (GroupNorm + single-head attention over 4096 tokens + residual)
on 8 Trainium2 NeuronCores.

Sharding: core c handles sample b=c//2 and query half h=c%2 (2048 queries).
The host rotates each sample's token axis so that each core's queries are
columns 0..2048 of its x_kv input (attention is permutation-invariant over
keys, and groupnorm stats are permutation-invariant), letting a single SPMD
program serve all 8 cores with no dynamic offsets.

Device-side math per core:
  h = groupnorm(x)                   fp32 stats (bn_stats + indicator-matmul
                                     cross-partition group reduce), fp16 output
  q = Wq h + bq  [256,2048] fp16     (host passes qkv_w^T; no on-chip transposes)
  k = Wk h + bk  [256,4096] fp16
  vT = (Wv h)^T  [4096,256] fp16     (computed directly in transposed layout)
  S^T tiles [128 keys, 512 queries] -> es = exp(S^T * C^-0.5) fp16
                                     (no max subtraction: |scores*scale| <~ 8)
  pv[c,q]  = sum_m vT[m,c] es[m,q]   (v chunks stationary, es streams, fp32 psum)
  den[q]   = sum_m es[m,q]           (ones-column matmul)
  fin      = out_w @ pv              (normalization deferred: a per-query column
                                      scale commutes through the projection)
  y = fin * (1/den bcast) + (out_w @ bv + ob) + x   (v-bias folded via sum(A)=1)

All matmuls run in fp16 (fp32 on the PE lowers to 2 HW passes at half rate);
PSUM accumulation is fp32 throughout, softmax statistics are fp32.
"""
import numpy as np

_CACHE: dict = {}

B, C, N = 4, 256, 4096
NQ = 2048          # queries per core
G = 8              # groups
GS = 32            # channels per group
EPS = 1e-5
SCALE = C ** -0.5


def _install_tile_drain_patch():
    """This container's walrus only supports ONE sync-wait command per
    instruction; TileContext's final drain carries one wait per live proc and
    fails codegen. Re-emit the waits as standalone sem-wait instructions."""
    import concourse.tile as tile_mod
    from concourse.vector_clock import ScopedClock

    if getattr(tile_mod.TileContext, "_ant_drain_patched", False):
        return

    def _patched(self, tick_clock, wait_clock):
        nc = self.nc
        probe = nc.sync.nop(nofuse=True)
        wait_clock.add_sem_waits(
            probe.ins, ScopedClock({None: tick_clock.global_clock})
        )
        waits = list(probe.ins.sync_info.on_wait or [])
        if len(waits) > 1:
            probe.ins.sync_info.on_wait = []
            num2h = {}
            assert self.sems is not None
            for h in self.sems.allocated().values():
                num2h[h.num] = h
            for h in self.sems.swdge_block():
                num2h.setdefault(h.num, h)
            for w in waits:
                h = num2h.get(w.id)
                assert h is not None, f"sem id {w.id} ({w.ant_name}) has no handle"
                nc.sync.wait_op(h, w.wait_value, "sem-ge")
        nc.sync.drain()
        nc.all_engine_barrier()
        assert self.sems is not None
        popped = nc._tile_sem_poison_stack.pop()
        assert popped is self._sem_poison
        nc.clear_and_free_semaphores(list(self.sems.allocated().values()))

    tile_mod.TileContext._drain_and_barrier = _patched
    tile_mod.TileContext._ant_drain_patched = True


def _split_multiwait_instructions(nc):
    """This walrus build supports only one sync-wait command per instruction.
    Hoist extra waits into standalone InstEventSemaphore waits on the same
    engine immediately before the instruction (sequential single waits are
    equivalent to one multi-wait)."""
    import concourse.mybir as mybir

    n = 0
    for f in nc.m.functions:
        for bb in f.blocks:
            new = []
            for ins in bb.instructions:
                si = ins.sync_info
                waits = list(si.on_wait) if si else []
                if len(waits) > 1:
                    for w in waits[:-1]:
                        nop = mybir.InstEventSemaphore(
                            name=f"WSPLIT-{n}", ins=[], outs=[])
                        n += 1
                        nop.engine = ins.engine
                        nop.sync_info = mybir.SyncInfo(on_wait=[w], on_update=[])
                        new.append(nop)
                    si.on_wait = [waits[-1]]
                new.append(ins)
            bb.instructions[:] = new


def _build():
    import concourse.bass as bass
    import concourse.tile as tile
    import concourse.mybir as mybir
    from contextlib import ExitStack

    _install_tile_drain_patch()

    f32 = mybir.dt.float32
    f16 = mybir.dt.float16
    Act = mybir.ActivationFunctionType
    Alu = mybir.AluOpType

    nc = bass.Bass()
    x_kv = nc.dram_tensor("x_kv", [C, N], f32, kind="ExternalInput")
    # w_pack = [qkv_wT | out_wT] : [256, 1024]; vec_pack cols =
    # qb0..qb5, nw0, nw1, nb0, nb1, ob0, ob1 : [128, 12]
    w_pack = nc.dram_tensor("w_pack", [C, 4 * C], f32, kind="ExternalInput")
    vec_pack = nc.dram_tensor("vec_pack", [128, 12], f32, kind="ExternalInput")
    y = nc.dram_tensor("y", [C, NQ], f32, kind="ExternalOutput")

    with ExitStack() as ctx:
        tc = ctx.enter_context(tile.TileContext(nc))
        const = ctx.enter_context(tc.tile_pool(name="const", bufs=1))
        xqp = ctx.enter_context(tc.tile_pool(name="xqp", bufs=16))
        hp = ctx.enter_context(tc.tile_pool(name="hp", bufs=16))
        kp = ctx.enter_context(tc.tile_pool(name="kp", bufs=2))
        qp = ctx.enter_context(tc.tile_pool(name="qp", bufs=2))
        vtp = ctx.enter_context(tc.tile_pool(name="vtp", bufs=32))
        esp = ctx.enter_context(tc.tile_pool(name="esp", bufs=16))
        smal = ctx.enter_context(tc.tile_pool(name="smal", bufs=2))
        sb2 = ctx.enter_context(tc.tile_pool(name="sb2", bufs=2))
        psp = ctx.enter_context(tc.tile_pool(name="psp", bufs=3, space="PSUM"))
        pvp = ctx.enter_context(tc.tile_pool(name="pvp", bufs=2, space="PSUM"))
        denp = ctx.enter_context(tc.tile_pool(name="denp", bufs=1, space="PSUM"))
        finp = ctx.enter_context(tc.tile_pool(name="finp", bufs=2, space="PSUM"))

        # ---- load x FIRST in [128,512] units round-robin over the three
        #      DMA-capable engines (one queue each); finer units mean the
        #      per-unit bn_stats pipelines tightly with arrivals ----
        xq = [[None] * 8 for _ in range(2)]
        dma_engs = [nc.sync, nc.gpsimd, nc.scalar]
        for u in range(8):
            for ct in range(2):
                t = xqp.tile([128, 512], f32, tag="xq", name=f"xq{ct}_{u}")
                eng = dma_engs[(u * 2 + ct) % 3]
                eng.dma_start(
                    out=t,
                    in_=x_kv[ct * 128:(ct + 1) * 128, u * 512:(u + 1) * 512])
                xq[ct][u] = t

        # ---- constants (after the x DMA issues: the tiny ind_bwd
        #      SBUF-to-SBUF DMAs would otherwise delay x on the sync queue) ----
        ones_col = const.tile([128, 128], f16, name="ones_col")
        nc.vector.memset(ones_col, 1.0)
        ones_row = const.tile([1, 128], f32, name="ones_row")
        nc.vector.memset(ones_row, 1.0)
        ind_fwd = const.tile([128, 4], f32, name="ind_fwd")   # [ch, grp] one-hot
        nc.vector.memset(ind_fwd, 0.0)
        for g in range(4):
            nc.vector.memset(ind_fwd[g * GS:(g + 1) * GS, g:g + 1], 1.0)
        ind_bwd = const.tile([4, 128], f32, name="ind_bwd")   # [grp, ch] one-hot
        nc.vector.memset(ind_bwd[0:4, :], 0.0)
        for g in range(4):
            # engines can't start at partition g∉{0,32,64,96}; DMA can
            nc.sync.dma_start(out=ind_bwd[g:g + 1, g * GS:(g + 1) * GS],
                              in_=ones_row[0:1, 0:GS])
        eps_t = const.tile([128, 1], f32, name="eps_t")
        nc.vector.memset(eps_t, EPS)

        # ---- packed weights / biases (2 + 1 DMA issues) ----
        wpk = []
        for ci in range(2):
            w = const.tile([128, 4 * C], f32, name=f"wpk{ci}")
            for hh in range(2):
                dma_engs[(ci * 2 + hh) % 3].dma_start(
                    out=w[:, hh * 512:(hh + 1) * 512],
                    in_=w_pack[ci * 128:(ci + 1) * 128,
                               hh * 512:(hh + 1) * 512])
            wpk.append(w)
        vp = const.tile([128, 12], f32, name="vp")
        nc.gpsimd.dma_start(out=vp, in_=vec_pack[:, :])
        wt = [wpk[ci][:, 0:3 * C] for ci in range(2)]
        owt = [wpk[ci][:, 3 * C:4 * C] for ci in range(2)]
        qb = [vp[:, j:j + 1] for j in range(6)]
        nw = [vp[:, 6 + ci:7 + ci] for ci in range(2)]
        nb = [vp[:, 8 + ci:9 + ci] for ci in range(2)]
        ob = [vp[:, 10 + ci:11 + ci] for ci in range(2)]

        # fp16 casts of matmul weights (fp32 on PE = 2 HW passes at half rate)
        wt16, owt16 = [], []
        for ci in range(2):
            w16 = const.tile([128, 3 * C], f16, name=f"wt16_{ci}")
            nc.vector.tensor_copy(out=w16, in_=wt[ci])
            wt16.append(w16)
            o16 = const.tile([128, C], f16, name=f"owt16_{ci}")
            nc.vector.tensor_copy(out=o16, in_=owt[ci])
            owt16.append(o16)
        qb16v = []
        for j in (4, 5):
            t16 = const.tile([128, 1], f16, name=f"qb16_{j}")
            nc.vector.tensor_copy(out=t16, in_=qb[j])
            qb16v.append(t16)

        # one bn_stats per arriving [128,512] unit (DVE)
        stats = [smal.tile([128, 8, 6], f32, tag=f"stats{ct}", name=f"stats{ct}")
                 for ct in range(2)]
        for u in range(8):
            for ct in range(2):
                nc.vector.bn_stats(out=stats[ct][:, u, :], in_=xq[ct][u])

        # paired layout: mv = [m0, m1, v0, v1] via strided aggr outputs, so
        # every chain op handles both ct tiles at once
        mv = smal.tile([128, 4], f32, tag="mv", name="mv0")
        nc.vector.bn_aggr(out=mv[:, 0:4:2], in_=stats[0])   # cols 0,2
        nc.vector.bn_aggr(out=mv[:, 1:4:2], in_=stats[1])   # cols 1,3
        st2 = smal.tile([128, 4], f32, tag="st2", name="st2")  # [m0,m1,E0,E1]
        nc.vector.tensor_copy(out=st2[:, 0:2], in_=mv[:, 0:2])
        nc.vector.tensor_mul(out=st2[:, 2:4], in0=mv[:, 0:2], in1=mv[:, 0:2])
        nc.vector.tensor_add(out=st2[:, 2:4], in0=st2[:, 2:4], in1=mv[:, 2:4])

        # cross-partition group reduce + broadcast back, both cts at once
        gp = psp.tile([4, 4], f32, tag="ps", name="gp")
        nc.tensor.matmul(gp, ind_fwd, st2, start=True, stop=True)
        gsb = smal.tile([4, 4], f32, tag="gsb", name="gsb")
        nc.scalar.mul(out=gsb, in_=gp, mul=1.0 / GS)
        chp = psp.tile([128, 4], f32, tag="ps", name="chp")
        nc.tensor.matmul(chp, ind_bwd, gsb, start=True, stop=True)
        ch = smal.tile([128, 4], f32, tag="ch", name="ch")   # [mu0,mu1,E20,E21]
        nc.vector.tensor_copy(out=ch, in_=chp)
        var = smal.tile([128, 2], f32, tag="var", name="var")
        nc.vector.tensor_mul(out=var, in0=ch[:, 0:2], in1=ch[:, 0:2])
        nc.vector.tensor_sub(out=var, in0=ch[:, 2:4], in1=var)
        std = smal.tile([128, 2], f32, tag="std", name="std")
        nc.scalar.activation(out=std, in_=var, func=Act.Sqrt, bias=eps_t)
        rstd = smal.tile([128, 2], f32, tag="rstd", name="rstd")
        nc.vector.reciprocal(out=rstd, in_=std)
        Aab = smal.tile([128, 2], f32, tag="Aab", name="Aab")
        nc.vector.tensor_mul(out=Aab, in0=vp[:, 6:8], in1=rstd)
        Bab = smal.tile([128, 2], f32, tag="Bab", name="Bab")
        nc.vector.tensor_mul(out=Bab, in0=ch[:, 0:2], in1=Aab)
        nc.vector.tensor_sub(out=Bab, in0=vp[:, 8:10], in1=Bab)
        AB = [(Aab[:, 0:1], Bab[:, 0:1]), (Aab[:, 1:2], Bab[:, 1:2])]

        # apply affine -> fp16 h tiles per 512 cols (ct0 on DVE, ct1 on
        # ACT, in parallel; fine granularity lets convs start per slice)
        h16 = [[None] * 8 for _ in range(2)]
        for s in range(8):
            for ct in range(2):
                ht = hp.tile([128, 512], f16, tag="h16", name=f"h16_{ct}_{s}")
                xsl = xq[ct][s]
                if ct == 0:
                    nc.vector.tensor_scalar(
                        out=ht, in0=xsl,
                        scalar1=AB[ct][0], scalar2=AB[ct][1],
                        op0=Alu.mult, op1=Alu.add)
                else:
                    nc.scalar.activation(
                        out=ht, in_=xsl, func=Act.Identity,
                        bias=AB[ct][1], scale=AB[ct][0])
                h16[ct][s] = ht

        def hsl(ct, nch):  # [128,512] tile for 512-col chunk nch
            return h16[ct][nch]

        def hmt(ct, mt):   # [128,128] slice of h16 for 128-col tile mt
            return h16[ct][mt // 4][:, (mt % 4) * 128:((mt % 4) + 1) * 128]

        # ---- q conv (queries = first NQ cols) ----
        q_sb = [qp.tile([128, NQ], f16, tag="q", name=f"q{co}") for co in range(2)]
        for co in range(2):
            for nch in range(NQ // 512):
                pq = psp.tile([128, 512], f32, tag="ps", name=f"pq{co}_{nch}")
                nc.tensor.matmul(pq, wt16[0][:, co * 128:(co + 1) * 128],
                                 hsl(0, nch), start=True, stop=False)
                nc.tensor.matmul(pq, wt16[1][:, co * 128:(co + 1) * 128],
                                 hsl(1, nch), start=False, stop=True)
                nc.vector.tensor_scalar_add(
                    out=q_sb[co][:, nch * 512:(nch + 1) * 512],
                    in0=pq, scalar1=qb[co])

        # ---- k conv ----
        k_sb = [kp.tile([128, N], f16, tag="k", name=f"k{co}") for co in range(2)]
        for co in range(2):
            for nch in range(N // 512):
                pk = psp.tile([128, 512], f32, tag="ps", name=f"pk{co}_{nch}")
                nc.tensor.matmul(pk, wt16[0][:, 256 + co * 128:256 + (co + 1) * 128],
                                 hsl(0, nch), start=True, stop=False)
                nc.tensor.matmul(pk, wt16[1][:, 256 + co * 128:256 + (co + 1) * 128],
                                 hsl(1, nch), start=False, stop=True)
                nc.vector.tensor_scalar_add(
                    out=k_sb[co][:, nch * 512:(nch + 1) * 512],
                    in0=pk, scalar1=qb[2 + co])

        # ---- vT conv (no bias; folded into effb) ----
        vt_sb = []
        for mt in range(32):
            pvt = psp.tile([128, 256], f32, tag="ps", name=f"pvt{mt}")
            nc.tensor.matmul(pvt, hmt(0, mt), wt16[0][:, 512:768],
                             start=True, stop=False)
            nc.tensor.matmul(pvt, hmt(1, mt), wt16[1][:, 512:768],
                             start=False, stop=True)
            vt = vtp.tile([128, 256], f16, tag="vt", name=f"vt{mt}")
            nc.vector.tensor_copy(out=vt, in_=pvt)
            vt_sb.append(vt)

        # ---- effective output bias: out_w @ v_bias + out_b ----
        effb = []
        for co in range(2):
            pe_ = psp.tile([128, 1], f32, tag="ps", name=f"peffb{co}")
            nc.tensor.matmul(pe_, owt16[0][:, co * 128:(co + 1) * 128], qb16v[0],
                             start=True, stop=False)
            nc.tensor.matmul(pe_, owt16[1][:, co * 128:(co + 1) * 128], qb16v[1],
                             start=False, stop=True)
            e = smal.tile([128, 1], f32, tag="effb", name=f"effb{co}")
            nc.vector.tensor_add(out=e, in0=pe_, in1=ob[co])
            effb.append(e)

        # ---- attention ----
        def st_chunk(t, mch):
            ps_s = psp.tile([128, 512], f32, tag="ps", name=f"s_{t}_{mch}")
            nc.tensor.matmul(ps_s, k_sb[0][:, mch * 128:(mch + 1) * 128],
                             q_sb[0][:, t * 512:(t + 1) * 512],
                             start=True, stop=False)
            nc.tensor.matmul(ps_s, k_sb[1][:, mch * 128:(mch + 1) * 128],
                             q_sb[1][:, t * 512:(t + 1) * 512],
                             start=False, stop=True)
            es = esp.tile([128, 512], f16, tag="es", name=f"es_{t}_{mch}")
            nc.scalar.activation(out=es, in_=ps_s, func=Act.Exp, scale=SCALE)
            return es

        def epilogue(t, pv, den):
            # den arrives already broadcast across partitions ([128,512]);
            # 1/den is entirely off the PE critical path (normalization
            # commutes through the output projection as a per-query scale).
            # The reciprocal and the y chain run in column halves so the
            # final tile's tail pipelines instead of serializing.
            rdb = sb2.tile([128, 512], f32, tag="rdb", name=f"rdb_{t}")
            for hh in range(2):
                nc.vector.reciprocal(out=rdb[:, hh * 256:(hh + 1) * 256],
                                     in_=den[:, hh * 256:(hh + 1) * 256])
            pvsb = []
            for co in range(2):
                p = sb2.tile([128, 512], f16, tag=f"pvsb{co}",
                             name=f"pvsb_{t}_{co}")
                nc.scalar.copy(out=p, in_=pv[co])  # frees the pv psum bank
                pvsb.append(p)
            fins, rsbs = [], []
            for co in range(2):
                fin = finp.tile([128, 512], f32, tag="fin", name=f"fin_{t}_{co}")
                nc.tensor.matmul(fin, owt16[0][:, co * 128:(co + 1) * 128],
                                 pvsb[0], start=True, stop=False)
                nc.tensor.matmul(fin, owt16[1][:, co * 128:(co + 1) * 128],
                                 pvsb[1], start=False, stop=True)
                rsb = sb2.tile([128, 512], f32, tag=f"rsb{co}", name=f"r_{t}_{co}")
                nc.sync.dma_start(
                    out=rsb,
                    in_=x_kv[co * 128:(co + 1) * 128, t * 512:(t + 1) * 512])
                fins.append(fin)
                rsbs.append(rsb)
            for co in range(2):
                t1 = sb2.tile([128, 512], f32, tag=f"t1{co}", name=f"t1_{t}_{co}")
                ysb = sb2.tile([128, 512], f32, tag=f"ysb{co}", name=f"y_{t}_{co}")
                for hh in range(2):
                    sl = slice(hh * 256, (hh + 1) * 256)
                    nc.vector.tensor_mul(out=t1[:, sl], in0=fins[co][:, sl],
                                         in1=rdb[:, sl])
                    nc.vector.scalar_tensor_tensor(
                        out=ysb[:, sl], in0=t1[:, sl], scalar=effb[co],
                        in1=rsbs[co][:, sl], op0=Alu.add, op1=Alu.add)
                    nc.sync.dma_start(
                        out=y[co * 128:(co + 1) * 128,
                              t * 512 + hh * 256:t * 512 + (hh + 1) * 256],
                        in_=ysb[:, sl])

        NT = NQ // 512
        pend = None
        for t in range(NT):
            # first two S^T chunks (and their exps) are emitted BEFORE the
            # previous tile's epilogue so the ACT stream does exp(t,0/1)
            # first and the PE never waits on the pvsb drains
            es_q = {m: st_chunk(t, m) for m in range(6)}
            if pend is not None:
                epilogue(*pend)   # frees the pv psum banks via pvsb copies
            pv = [pvp.tile([128, 512], f32, tag="pv", name=f"pv_{t}_{co}")
                  for co in range(2)]
            den = denp.tile([128, 512], f32, tag="den", name=f"den_{t}")
            grp, e2s = [], []
            pend_den, den_started = None, False

            def emit_den(rhs, stop=False):
                nonlocal den_started
                nc.tensor.matmul(den, ones_col, rhs,
                                 start=not den_started, stop=stop)
                den_started = True

            for mch in range(32):
                es = es_q.pop(mch)
                st, sp = (mch == 0), (mch == 31)
                nc.tensor.matmul(pv[0], vt_sb[mch][:, 0:128], es, start=st, stop=sp)
                nc.tensor.matmul(pv[1], vt_sb[mch][:, 128:256], es, start=st, stop=sp)
                if mch == 28 and pend_den is not None:
                    emit_den(pend_den)
                    pend_den = None
                if mch >= 28:
                    # tail chunks feed den directly: no GpSimd-add latency at
                    # the tile boundary (den gates the epilogue + next tile)
                    emit_den(es, stop=(mch == 31))
                else:
                    # es quad-sums on the otherwise idle GpSimd engine
                    # quarter the den matmul streams through the PE; each
                    # quad's den matmul is deferred one quad so the PE never
                    # waits on the 3-add GpSimd chain
                    grp.append(es)
                    if len(grp) == 2:
                        e2 = esp.tile([128, 512], f16, tag="es2",
                                      name=f"es2_{t}_{mch}")
                        nc.gpsimd.tensor_add(out=e2, in0=grp[0], in1=grp[1])
                        grp, e2s = [], e2s + [e2]
                        if len(e2s) == 2:
                            q4 = mch // 4
                            e4 = esp.tile([128, 512], f16, tag="es4",
                                          name=f"es4_{t}_{q4}")
                            nc.gpsimd.tensor_add(out=e4, in0=e2s[0], in1=e2s[1])
                            e2s = []
                            if pend_den is not None:
                                emit_den(pend_den)
                            pend_den = e4
                if mch + 6 < 32:
                    es_q[mch + 6] = st_chunk(t, mch + 6)
            pend = (t, pv, den)
        epilogue(*pend)

    _split_multiwait_instructions(nc)
    return nc


def _get_nc():
    if "nc" not in _CACHE:
        _CACHE["nc"] = _build()
    return _CACHE["nc"]


def _prep_in_maps(x, norm_w, norm_b, qkv_w, qkv_b, out_w, out_b):
    xr = np.ascontiguousarray(np.asarray(x, dtype=np.float32).reshape(B, C, N))
    wT = np.asarray(qkv_w, np.float32).T
    owT = np.asarray(out_w, np.float32).T
    w_pack = np.ascontiguousarray(np.concatenate([wT, owT], axis=1))
    qb1 = np.asarray(qkv_b, np.float32)
    vec_pack = np.ascontiguousarray(np.stack(
        [qb1[0:128], qb1[128:256], qb1[256:384], qb1[384:512],
         qb1[512:640], qb1[640:768],
         np.asarray(norm_w, np.float32)[0:128],
         np.asarray(norm_w, np.float32)[128:256],
         np.asarray(norm_b, np.float32)[0:128],
         np.asarray(norm_b, np.float32)[128:256],
         np.asarray(out_b, np.float32)[0:128],
         np.asarray(out_b, np.float32)[128:256]], axis=1))

    in_maps = []
    for c in range(8):
        b, h = divmod(c, 2)
        off = h * NQ
        xb = xr[b]
        if off:
            xroll = np.ascontiguousarray(
                np.concatenate([xb[:, off:], xb[:, :off]], axis=1))
        else:
            xroll = xb
        in_maps.append({"x_kv": xroll, "w_pack": w_pack, "vec_pack": vec_pack})
    return in_maps


def kernel(x, norm_w, norm_b, qkv_w, qkv_b, out_w, out_b):
    from concourse.bass_utils import run_bass_kernel_spmd

    nc = _get_nc()
    in_maps = _prep_in_maps(x, norm_w, norm_b, qkv_w, qkv_b, out_w, out_b)
    res = run_bass_kernel_spmd(nc, in_maps, core_ids=list(range(8)))
    out = np.empty((B, C, N), np.float32)
    for c in range(8):
        b, h = divmod(c, 2)
        off = h * NQ
        out[b][:, off:off + NQ] = res.results[c]["y"]
    return out.reshape(B, C, 16, 16, 16)


# revision 34
# speedup vs baseline: 1.0007x; 1.0003x over previous
"""AttentionBlock3D (GroupNorm + single-head attention over 4096 tokens + residual)
on 8 Trainium2 NeuronCores.

Sharding: core c handles sample b=c//2 and query half h=c%2 (2048 queries).
The host rotates each sample's token axis so that each core's queries are
columns 0..2048 of its x_kv input (attention is permutation-invariant over
keys, and groupnorm stats are permutation-invariant), letting a single SPMD
program serve all 8 cores with no dynamic offsets.

Device-side math per core:
  h = groupnorm(x)                   fp32 stats (bn_stats + indicator-matmul
                                     cross-partition group reduce), fp16 output
  q = Wq h + bq  [256,2048] fp16     (host passes qkv_w^T; no on-chip transposes)
  k = Wk h + bk  [256,4096] fp16
  vT = (Wv h)^T  [4096,256] fp16     (computed directly in transposed layout)
  S^T tiles [128 keys, 512 queries] -> es = exp(S^T * C^-0.5) fp16
                                     (no max subtraction: |scores*scale| <~ 8)
  pv[c,q]  = sum_m vT[m,c] es[m,q]   (v chunks stationary, es streams, fp32 psum)
  den[q]   = sum_m es[m,q]           (ones-column matmul)
  fin      = out_w @ pv              (normalization deferred: a per-query column
                                      scale commutes through the projection)
  y = fin * (1/den bcast) + (out_w @ bv + ob) + x   (v-bias folded via sum(A)=1)

All matmuls run in fp16 (fp32 on the PE lowers to 2 HW passes at half rate);
PSUM accumulation is fp32 throughout, softmax statistics are fp32.
"""
import numpy as np

_CACHE: dict = {}

B, C, N = 4, 256, 4096
NQ = 2048          # queries per core
G = 8              # groups
GS = 32            # channels per group
EPS = 1e-5
SCALE = C ** -0.5


def _install_tile_drain_patch():
    """This container's walrus only supports ONE sync-wait command per
    instruction; TileContext's final drain carries one wait per live proc and
    fails codegen. Re-emit the waits as standalone sem-wait instructions."""
    import concourse.tile as tile_mod
    from concourse.vector_clock import ScopedClock

    if getattr(tile_mod.TileContext, "_ant_drain_patched", False):
        return

    def _patched(self, tick_clock, wait_clock):
        nc = self.nc
        probe = nc.sync.nop(nofuse=True)
        wait_clock.add_sem_waits(
            probe.ins, ScopedClock({None: tick_clock.global_clock})
        )
        waits = list(probe.ins.sync_info.on_wait or [])
        if len(waits) > 1:
            probe.ins.sync_info.on_wait = []
            num2h = {}
            assert self.sems is not None
            for h in self.sems.allocated().values():
                num2h[h.num] = h
            for h in self.sems.swdge_block():
                num2h.setdefault(h.num, h)
            for w in waits:
                h = num2h.get(w.id)
                assert h is not None, f"sem id {w.id} ({w.ant_name}) has no handle"
                nc.sync.wait_op(h, w.wait_value, "sem-ge")
        nc.sync.drain()
        nc.all_engine_barrier()
        assert self.sems is not None
        popped = nc._tile_sem_poison_stack.pop()
        assert popped is self._sem_poison
        nc.clear_and_free_semaphores(list(self.sems.allocated().values()))

    tile_mod.TileContext._drain_and_barrier = _patched
    tile_mod.TileContext._ant_drain_patched = True


def _split_multiwait_instructions(nc):
    """This walrus build supports only one sync-wait command per instruction.
    Hoist extra waits into standalone InstEventSemaphore waits on the same
    engine immediately before the instruction (sequential single waits are
    equivalent to one multi-wait)."""
    import concourse.mybir as mybir

    n = 0
    for f in nc.m.functions:
        for bb in f.blocks:
            new = []
            for ins in bb.instructions:
                si = ins.sync_info
                waits = list(si.on_wait) if si else []
                if len(waits) > 1:
                    for w in waits[:-1]:
                        nop = mybir.InstEventSemaphore(
                            name=f"WSPLIT-{n}", ins=[], outs=[])
                        n += 1
                        nop.engine = ins.engine
                        nop.sync_info = mybir.SyncInfo(on_wait=[w], on_update=[])
                        new.append(nop)
                    si.on_wait = [waits[-1]]
                new.append(ins)
            bb.instructions[:] = new


def _build():
    import concourse.bass as bass
    import concourse.tile as tile
    import concourse.mybir as mybir
    from contextlib import ExitStack

    _install_tile_drain_patch()

    f32 = mybir.dt.float32
    f16 = mybir.dt.float16
    Act = mybir.ActivationFunctionType
    Alu = mybir.AluOpType

    nc = bass.Bass()
    x_kv = nc.dram_tensor("x_kv", [C, N], f32, kind="ExternalInput")
    # w_pack = [qkv_wT | out_wT] : [256, 1024]; vec_pack cols =
    # qb0..qb5, nw0, nw1, nb0, nb1, ob0, ob1 : [128, 12]
    w_pack = nc.dram_tensor("w_pack", [C, 4 * C], f32, kind="ExternalInput")
    vec_pack = nc.dram_tensor("vec_pack", [128, 12], f32, kind="ExternalInput")
    y = nc.dram_tensor("y", [C, NQ], f32, kind="ExternalOutput")

    with ExitStack() as ctx:
        tc = ctx.enter_context(tile.TileContext(nc))
        const = ctx.enter_context(tc.tile_pool(name="const", bufs=1))
        xqp = ctx.enter_context(tc.tile_pool(name="xqp", bufs=16))
        hp = ctx.enter_context(tc.tile_pool(name="hp", bufs=16))
        kp = ctx.enter_context(tc.tile_pool(name="kp", bufs=2))
        qp = ctx.enter_context(tc.tile_pool(name="qp", bufs=2))
        vtp = ctx.enter_context(tc.tile_pool(name="vtp", bufs=32))
        esp = ctx.enter_context(tc.tile_pool(name="esp", bufs=16))
        smal = ctx.enter_context(tc.tile_pool(name="smal", bufs=2))
        sb2 = ctx.enter_context(tc.tile_pool(name="sb2", bufs=2))
        psp = ctx.enter_context(tc.tile_pool(name="psp", bufs=3, space="PSUM"))
        pvp = ctx.enter_context(tc.tile_pool(name="pvp", bufs=2, space="PSUM"))
        denp = ctx.enter_context(tc.tile_pool(name="denp", bufs=1, space="PSUM"))
        finp = ctx.enter_context(tc.tile_pool(name="finp", bufs=2, space="PSUM"))

        # ---- load x FIRST in [128,512] units round-robin over the three
        #      DMA-capable engines (one queue each); finer units mean the
        #      per-unit bn_stats pipelines tightly with arrivals ----
        xq = [[None] * 8 for _ in range(2)]
        dma_engs = [nc.sync, nc.gpsimd, nc.scalar]
        for u in range(8):
            for ct in range(2):
                t = xqp.tile([128, 512], f32, tag="xq", name=f"xq{ct}_{u}")
                eng = dma_engs[(u * 2 + ct) % 3]
                eng.dma_start(
                    out=t,
                    in_=x_kv[ct * 128:(ct + 1) * 128, u * 512:(u + 1) * 512])
                xq[ct][u] = t

        # ---- constants (after the x DMA issues: the tiny ind_bwd
        #      SBUF-to-SBUF DMAs would otherwise delay x on the sync queue) ----
        ones_col = const.tile([128, 128], f16, name="ones_col")
        nc.vector.memset(ones_col, 1.0)
        ones_row = const.tile([1, 128], f32, name="ones_row")
        nc.vector.memset(ones_row, 1.0)
        ind_fwd = const.tile([128, 4], f32, name="ind_fwd")   # [ch, grp] one-hot
        nc.vector.memset(ind_fwd, 0.0)
        for g in range(4):
            nc.vector.memset(ind_fwd[g * GS:(g + 1) * GS, g:g + 1], 1.0)
        ind_bwd = const.tile([4, 128], f32, name="ind_bwd")   # [grp, ch] one-hot
        nc.vector.memset(ind_bwd[0:4, :], 0.0)
        for g in range(4):
            # engines can't start at partition g∉{0,32,64,96}; DMA can
            nc.sync.dma_start(out=ind_bwd[g:g + 1, g * GS:(g + 1) * GS],
                              in_=ones_row[0:1, 0:GS])
        eps_t = const.tile([128, 1], f32, name="eps_t")
        nc.vector.memset(eps_t, EPS)

        # ---- packed weights / biases (2 + 1 DMA issues) ----
        wpk = []
        for ci in range(2):
            w = const.tile([128, 4 * C], f32, name=f"wpk{ci}")
            for hh in range(2):
                dma_engs[(ci * 2 + hh) % 3].dma_start(
                    out=w[:, hh * 512:(hh + 1) * 512],
                    in_=w_pack[ci * 128:(ci + 1) * 128,
                               hh * 512:(hh + 1) * 512])
            wpk.append(w)
        vp = const.tile([128, 12], f32, name="vp")
        nc.gpsimd.dma_start(out=vp, in_=vec_pack[:, :])
        wt = [wpk[ci][:, 0:3 * C] for ci in range(2)]
        owt = [wpk[ci][:, 3 * C:4 * C] for ci in range(2)]
        qb = [vp[:, j:j + 1] for j in range(6)]
        nw = [vp[:, 6 + ci:7 + ci] for ci in range(2)]
        nb = [vp[:, 8 + ci:9 + ci] for ci in range(2)]
        ob = [vp[:, 10 + ci:11 + ci] for ci in range(2)]

        # fp16 casts of matmul weights (fp32 on PE = 2 HW passes at half rate)
        wt16, owt16 = [], []
        for ci in range(2):
            w16 = const.tile([128, 3 * C], f16, name=f"wt16_{ci}")
            nc.vector.tensor_copy(out=w16, in_=wt[ci])
            wt16.append(w16)
            o16 = const.tile([128, C], f16, name=f"owt16_{ci}")
            nc.vector.tensor_copy(out=o16, in_=owt[ci])
            owt16.append(o16)
        qb16v = []
        for j in (4, 5):
            t16 = const.tile([128, 1], f16, name=f"qb16_{j}")
            nc.vector.tensor_copy(out=t16, in_=qb[j])
            qb16v.append(t16)

        # one bn_stats per arriving [128,512] unit (DVE)
        stats = [smal.tile([128, 8, 6], f32, tag=f"stats{ct}", name=f"stats{ct}")
                 for ct in range(2)]
        for u in range(8):
            for ct in range(2):
                nc.vector.bn_stats(out=stats[ct][:, u, :], in_=xq[ct][u])

        # paired layout: mv = [m0, m1, v0, v1] via strided aggr outputs, so
        # every chain op handles both ct tiles at once
        mv = smal.tile([128, 4], f32, tag="mv", name="mv0")
        nc.vector.bn_aggr(out=mv[:, 0:4:2], in_=stats[0])   # cols 0,2
        nc.vector.bn_aggr(out=mv[:, 1:4:2], in_=stats[1])   # cols 1,3
        st2 = smal.tile([128, 4], f32, tag="st2", name="st2")  # [m0,m1,E0,E1]
        nc.vector.tensor_copy(out=st2[:, 0:2], in_=mv[:, 0:2])
        nc.vector.tensor_mul(out=st2[:, 2:4], in0=mv[:, 0:2], in1=mv[:, 0:2])
        nc.vector.tensor_add(out=st2[:, 2:4], in0=st2[:, 2:4], in1=mv[:, 2:4])

        # cross-partition group reduce + broadcast back, both cts at once
        gp = psp.tile([4, 4], f32, tag="ps", name="gp")
        nc.tensor.matmul(gp, ind_fwd, st2, start=True, stop=True)
        gsb = smal.tile([4, 4], f32, tag="gsb", name="gsb")
        nc.scalar.mul(out=gsb, in_=gp, mul=1.0 / GS)
        chp = psp.tile([128, 4], f32, tag="ps", name="chp")
        nc.tensor.matmul(chp, ind_bwd, gsb, start=True, stop=True)
        ch = smal.tile([128, 4], f32, tag="ch", name="ch")   # [mu0,mu1,E20,E21]
        nc.vector.tensor_copy(out=ch, in_=chp)
        var = smal.tile([128, 2], f32, tag="var", name="var")
        nc.vector.tensor_mul(out=var, in0=ch[:, 0:2], in1=ch[:, 0:2])
        nc.vector.tensor_sub(out=var, in0=ch[:, 2:4], in1=var)
        std = smal.tile([128, 2], f32, tag="std", name="std")
        nc.scalar.activation(out=std, in_=var, func=Act.Sqrt, bias=eps_t)
        rstd = smal.tile([128, 2], f32, tag="rstd", name="rstd")
        nc.vector.reciprocal(out=rstd, in_=std)
        Aab = smal.tile([128, 2], f32, tag="Aab", name="Aab")
        nc.vector.tensor_mul(out=Aab, in0=vp[:, 6:8], in1=rstd)
        Bab = smal.tile([128, 2], f32, tag="Bab", name="Bab")
        nc.vector.tensor_mul(out=Bab, in0=ch[:, 0:2], in1=Aab)
        nc.vector.tensor_sub(out=Bab, in0=vp[:, 8:10], in1=Bab)
        AB = [(Aab[:, 0:1], Bab[:, 0:1]), (Aab[:, 1:2], Bab[:, 1:2])]

        # apply affine -> fp16 h tiles per 512 cols (ct0 on DVE, ct1 on
        # ACT, in parallel; fine granularity lets convs start per slice)
        h16 = [[None] * 8 for _ in range(2)]
        for s in range(8):
            for ct in range(2):
                ht = hp.tile([128, 512], f16, tag="h16", name=f"h16_{ct}_{s}")
                xsl = xq[ct][s]
                if ct == 0:
                    nc.vector.tensor_scalar(
                        out=ht, in0=xsl,
                        scalar1=AB[ct][0], scalar2=AB[ct][1],
                        op0=Alu.mult, op1=Alu.add)
                else:
                    nc.scalar.activation(
                        out=ht, in_=xsl, func=Act.Identity,
                        bias=AB[ct][1], scale=AB[ct][0])
                h16[ct][s] = ht

        def hsl(ct, nch):  # [128,512] tile for 512-col chunk nch
            return h16[ct][nch]

        def hmt(ct, mt):   # [128,128] slice of h16 for 128-col tile mt
            return h16[ct][mt // 4][:, (mt % 4) * 128:((mt % 4) + 1) * 128]

        # ---- q conv (queries = first NQ cols) ----
        q_sb = [qp.tile([128, NQ], f16, tag="q", name=f"q{co}") for co in range(2)]
        for co in range(2):
            for nch in range(NQ // 512):
                pq = psp.tile([128, 512], f32, tag="ps", name=f"pq{co}_{nch}")
                nc.tensor.matmul(pq, wt16[0][:, co * 128:(co + 1) * 128],
                                 hsl(0, nch), start=True, stop=False)
                nc.tensor.matmul(pq, wt16[1][:, co * 128:(co + 1) * 128],
                                 hsl(1, nch), start=False, stop=True)
                nc.vector.tensor_scalar_add(
                    out=q_sb[co][:, nch * 512:(nch + 1) * 512],
                    in0=pq, scalar1=qb[co])

        # ---- k conv ----
        k_sb = [kp.tile([128, N], f16, tag="k", name=f"k{co}") for co in range(2)]
        for co in range(2):
            for nch in range(N // 512):
                pk = psp.tile([128, 512], f32, tag="ps", name=f"pk{co}_{nch}")
                nc.tensor.matmul(pk, wt16[0][:, 256 + co * 128:256 + (co + 1) * 128],
                                 hsl(0, nch), start=True, stop=False)
                nc.tensor.matmul(pk, wt16[1][:, 256 + co * 128:256 + (co + 1) * 128],
                                 hsl(1, nch), start=False, stop=True)
                nc.vector.tensor_scalar_add(
                    out=k_sb[co][:, nch * 512:(nch + 1) * 512],
                    in0=pk, scalar1=qb[2 + co])

        # ---- vT conv (no bias; folded into effb) ----
        vt_sb = []
        for mt in range(32):
            pvt = psp.tile([128, 256], f32, tag="ps", name=f"pvt{mt}")
            nc.tensor.matmul(pvt, hmt(0, mt), wt16[0][:, 512:768],
                             start=True, stop=False)
            nc.tensor.matmul(pvt, hmt(1, mt), wt16[1][:, 512:768],
                             start=False, stop=True)
            vt = vtp.tile([128, 256], f16, tag="vt", name=f"vt{mt}")
            nc.vector.tensor_copy(out=vt, in_=pvt)
            vt_sb.append(vt)

        # ---- effective output bias: out_w @ v_bias + out_b ----
        effb = []
        for co in range(2):
            pe_ = psp.tile([128, 1], f32, tag="ps", name=f"peffb{co}")
            nc.tensor.matmul(pe_, owt16[0][:, co * 128:(co + 1) * 128], qb16v[0],
                             start=True, stop=False)
            nc.tensor.matmul(pe_, owt16[1][:, co * 128:(co + 1) * 128], qb16v[1],
                             start=False, stop=True)
            e = smal.tile([128, 1], f32, tag="effb", name=f"effb{co}")
            nc.vector.tensor_add(out=e, in0=pe_, in1=ob[co])
            effb.append(e)

        # ---- attention ----
        def st_chunk(t, mch):
            ps_s = psp.tile([128, 512], f32, tag="ps", name=f"s_{t}_{mch}")
            nc.tensor.matmul(ps_s, k_sb[0][:, mch * 128:(mch + 1) * 128],
                             q_sb[0][:, t * 512:(t + 1) * 512],
                             start=True, stop=False)
            nc.tensor.matmul(ps_s, k_sb[1][:, mch * 128:(mch + 1) * 128],
                             q_sb[1][:, t * 512:(t + 1) * 512],
                             start=False, stop=True)
            es = esp.tile([128, 512], f16, tag="es", name=f"es_{t}_{mch}")
            nc.scalar.activation(out=es, in_=ps_s, func=Act.Exp, scale=SCALE)
            return es

        def epilogue(t, pv, den):
            # den arrives already broadcast across partitions ([128,512]);
            # 1/den is entirely off the PE critical path (normalization
            # commutes through the output projection as a per-query scale).
            # The reciprocal and the y chain run in column halves so the
            # final tile's tail pipelines instead of serializing.
            rdb = sb2.tile([128, 512], f32, tag="rdb", name=f"rdb_{t}")
            for hh in range(2):
                nc.vector.reciprocal(out=rdb[:, hh * 256:(hh + 1) * 256],
                                     in_=den[:, hh * 256:(hh + 1) * 256])
            pvsb = []
            for co in range(2):
                p = sb2.tile([128, 512], f16, tag=f"pvsb{co}",
                             name=f"pvsb_{t}_{co}")
                nc.scalar.copy(out=p, in_=pv[co])  # frees the pv psum bank
                pvsb.append(p)
            fins, rsbs = [], []
            for co in range(2):
                fin = finp.tile([128, 512], f32, tag="fin", name=f"fin_{t}_{co}")
                nc.tensor.matmul(fin, owt16[0][:, co * 128:(co + 1) * 128],
                                 pvsb[0], start=True, stop=False)
                nc.tensor.matmul(fin, owt16[1][:, co * 128:(co + 1) * 128],
                                 pvsb[1], start=False, stop=True)
                rsb = sb2.tile([128, 512], f32, tag=f"rsb{co}", name=f"r_{t}_{co}")
                nc.sync.dma_start(
                    out=rsb,
                    in_=x_kv[co * 128:(co + 1) * 128, t * 512:(t + 1) * 512])
                fins.append(fin)
                rsbs.append(rsb)
            for co in range(2):
                t1 = sb2.tile([128, 512], f32, tag=f"t1{co}", name=f"t1_{t}_{co}")
                ysb = sb2.tile([128, 512], f32, tag=f"ysb{co}", name=f"y_{t}_{co}")
                for hh in range(2):
                    sl = slice(hh * 256, (hh + 1) * 256)
                    nc.vector.tensor_mul(out=t1[:, sl], in0=fins[co][:, sl],
                                         in1=rdb[:, sl])
                    nc.vector.scalar_tensor_tensor(
                        out=ysb[:, sl], in0=t1[:, sl], scalar=effb[co],
                        in1=rsbs[co][:, sl], op0=Alu.add, op1=Alu.add)
                    nc.sync.dma_start(
                        out=y[co * 128:(co + 1) * 128,
                              t * 512 + hh * 256:t * 512 + (hh + 1) * 256],
                        in_=ysb[:, sl])

        NT = NQ // 512
        pend = None
        for t in range(NT):
            # first two S^T chunks (and their exps) are emitted BEFORE the
            # previous tile's epilogue so the ACT stream does exp(t,0/1)
            # first and the PE never waits on the pvsb drains
            es_q = {m: st_chunk(t, m) for m in range(4)}
            if pend is not None:
                epilogue(*pend)   # frees the pv psum banks via pvsb copies
            pv = [pvp.tile([128, 512], f32, tag="pv", name=f"pv_{t}_{co}")
                  for co in range(2)]
            den = denp.tile([128, 512], f32, tag="den", name=f"den_{t}")
            grp, e2s = [], []
            pend_den, den_started = None, False

            def emit_den(rhs, stop=False):
                nonlocal den_started
                nc.tensor.matmul(den, ones_col, rhs,
                                 start=not den_started, stop=stop)
                den_started = True

            for mch in range(32):
                es = es_q.pop(mch)
                st, sp = (mch == 0), (mch == 31)
                nc.tensor.matmul(pv[0], vt_sb[mch][:, 0:128], es, start=st, stop=sp)
                nc.tensor.matmul(pv[1], vt_sb[mch][:, 128:256], es, start=st, stop=sp)
                if mch == 28 and pend_den is not None:
                    emit_den(pend_den)
                    pend_den = None
                if mch >= 28:
                    # tail chunks feed den directly: no GpSimd-add latency at
                    # the tile boundary (den gates the epilogue + next tile)
                    emit_den(es, stop=(mch == 31))
                else:
                    # es quad-sums on the otherwise idle GpSimd engine
                    # quarter the den matmul streams through the PE; each
                    # quad's den matmul is deferred one quad so the PE never
                    # waits on the 3-add GpSimd chain
                    grp.append(es)
                    if len(grp) == 2:
                        e2 = esp.tile([128, 512], f16, tag="es2",
                                      name=f"es2_{t}_{mch}")
                        nc.gpsimd.tensor_add(out=e2, in0=grp[0], in1=grp[1])
                        grp, e2s = [], e2s + [e2]
                        if len(e2s) == 2:
                            q4 = mch // 4
                            e4 = esp.tile([128, 512], f16, tag="es4",
                                          name=f"es4_{t}_{q4}")
                            nc.gpsimd.tensor_add(out=e4, in0=e2s[0], in1=e2s[1])
                            e2s = []
                            if pend_den is not None:
                                emit_den(pend_den)
                            pend_den = e4
                if mch + 4 < 32:
                    es_q[mch + 4] = st_chunk(t, mch + 4)
            pend = (t, pv, den)
        epilogue(*pend)

    _split_multiwait_instructions(nc)
    return nc


def _get_nc():
    if "nc" not in _CACHE:
        _CACHE["nc"] = _build()
    return _CACHE["nc"]


def _prep_in_maps(x, norm_w, norm_b, qkv_w, qkv_b, out_w, out_b):
    xr = np.ascontiguousarray(np.asarray(x, dtype=np.float32).reshape(B, C, N))
    wT = np.asarray(qkv_w, np.float32).T
    owT = np.asarray(out_w, np.float32).T
    w_pack = np.ascontiguousarray(np.concatenate([wT, owT], axis=1))
    qb1 = np.asarray(qkv_b, np.float32)
    vec_pack = np.ascontiguousarray(np.stack(
        [qb1[0:128], qb1[128:256], qb1[256:384], qb1[384:512],
         qb1[512:640], qb1[640:768],
         np.asarray(norm_w, np.float32)[0:128],
         np.asarray(norm_w, np.float32)[128:256],
         np.asarray(norm_b, np.float32)[0:128],
         np.asarray(norm_b, np.float32)[128:256],
         np.asarray(out_b, np.float32)[0:128],
         np.asarray(out_b, np.float32)[128:256]], axis=1))

    in_maps = []
    for c in range(8):
        b, h = divmod(c, 2)
        off = h * NQ
        xb = xr[b]
        if off:
            xroll = np.ascontiguousarray(
                np.concatenate([xb[:, off:], xb[:, :off]], axis=1))
        else:
            xroll = xb
        in_maps.append({"x_kv": xroll, "w_pack": w_pack, "vec_pack": vec_pack})
    return in_maps


def kernel(x, norm_w, norm_b, qkv_w, qkv_b, out_w, out_b):
    from concourse.bass_utils import run_bass_kernel_spmd

    nc = _get_nc()
    in_maps = _prep_in_maps(x, norm_w, norm_b, qkv_w, qkv_b, out_w, out_b)
    res = run_bass_kernel_spmd(nc, in_maps, core_ids=list(range(8)))
    out = np.empty((B, C, N), np.float32)
    for c in range(8):
        b, h = divmod(c, 2)
        off = h * NQ
        out[b][:, off:off + NQ] = res.results[c]["y"]
    return out.reshape(B, C, 16, 16, 16)


# revision 35
# speedup vs baseline: 1.0135x; 1.0128x over previous
"""AttentionBlock3D (GroupNorm + single-head attention over 4096 tokens + residual)
on 8 Trainium2 NeuronCores.

Sharding: core c handles sample b=c//2 and query half h=c%2 (2048 queries).
The host rotates each sample's token axis so that each core's queries are
columns 0..2048 of its x_kv input (attention is permutation-invariant over
keys, and groupnorm stats are permutation-invariant), letting a single SPMD
program serve all 8 cores with no dynamic offsets.

Device-side math per core:
  h = groupnorm(x)                   fp32 stats (bn_stats + indicator-matmul
                                     cross-partition group reduce), fp16 output
  q = Wq h + bq  [256,2048] fp16     (host passes qkv_w^T; no on-chip transposes)
  k = Wk h + bk  [256,4096] fp16
  vT = (Wv h)^T  [4096,256] fp16     (computed directly in transposed layout)
  S^T tiles [128 keys, 512 queries] -> es = exp(S^T * C^-0.5) fp16
                                     (no max subtraction: |scores*scale| <~ 8)
  pv[c,q]  = sum_m vT[m,c] es[m,q]   (v chunks stationary, es streams, fp32 psum)
  den[q]   = sum_m es[m,q]           (ones-column matmul)
  fin      = out_w @ pv              (normalization deferred: a per-query column
                                      scale commutes through the projection)
  y = fin * (1/den bcast) + (out_w @ bv + ob) + x   (v-bias folded via sum(A)=1)

All matmuls run in fp16 (fp32 on the PE lowers to 2 HW passes at half rate);
PSUM accumulation is fp32 throughout, softmax statistics are fp32.
"""
import numpy as np

_CACHE: dict = {}

B, C, N = 4, 256, 4096
NQ = 2048          # queries per core
G = 8              # groups
GS = 32            # channels per group
EPS = 1e-5
SCALE = C ** -0.5


def _install_tile_drain_patch():
    """This container's walrus only supports ONE sync-wait command per
    instruction; TileContext's final drain carries one wait per live proc and
    fails codegen. Re-emit the waits as standalone sem-wait instructions."""
    import concourse.tile as tile_mod
    from concourse.vector_clock import ScopedClock

    if getattr(tile_mod.TileContext, "_ant_drain_patched", False):
        return

    def _patched(self, tick_clock, wait_clock):
        nc = self.nc
        probe = nc.sync.nop(nofuse=True)
        wait_clock.add_sem_waits(
            probe.ins, ScopedClock({None: tick_clock.global_clock})
        )
        waits = list(probe.ins.sync_info.on_wait or [])
        if len(waits) > 1:
            probe.ins.sync_info.on_wait = []
            num2h = {}
            assert self.sems is not None
            for h in self.sems.allocated().values():
                num2h[h.num] = h
            for h in self.sems.swdge_block():
                num2h.setdefault(h.num, h)
            for w in waits:
                h = num2h.get(w.id)
                assert h is not None, f"sem id {w.id} ({w.ant_name}) has no handle"
                nc.sync.wait_op(h, w.wait_value, "sem-ge")
        nc.sync.drain()
        nc.all_engine_barrier()
        assert self.sems is not None
        popped = nc._tile_sem_poison_stack.pop()
        assert popped is self._sem_poison
        nc.clear_and_free_semaphores(list(self.sems.allocated().values()))

    tile_mod.TileContext._drain_and_barrier = _patched
    tile_mod.TileContext._ant_drain_patched = True


def _split_multiwait_instructions(nc):
    """This walrus build supports only one sync-wait command per instruction.
    Hoist extra waits into standalone InstEventSemaphore waits on the same
    engine immediately before the instruction (sequential single waits are
    equivalent to one multi-wait)."""
    import concourse.mybir as mybir

    n = 0
    for f in nc.m.functions:
        for bb in f.blocks:
            new = []
            for ins in bb.instructions:
                si = ins.sync_info
                waits = list(si.on_wait) if si else []
                if len(waits) > 1:
                    for w in waits[:-1]:
                        nop = mybir.InstEventSemaphore(
                            name=f"WSPLIT-{n}", ins=[], outs=[])
                        n += 1
                        nop.engine = ins.engine
                        nop.sync_info = mybir.SyncInfo(on_wait=[w], on_update=[])
                        new.append(nop)
                    si.on_wait = [waits[-1]]
                new.append(ins)
            bb.instructions[:] = new


def _build():
    import concourse.bass as bass
    import concourse.tile as tile
    import concourse.mybir as mybir
    from contextlib import ExitStack

    _install_tile_drain_patch()

    f32 = mybir.dt.float32
    f16 = mybir.dt.float16
    Act = mybir.ActivationFunctionType
    Alu = mybir.AluOpType

    nc = bass.Bass()
    x_kv = nc.dram_tensor("x_kv", [C, N], f32, kind="ExternalInput")
    x16 = nc.dram_tensor("x16", [C, N], f16, kind="ExternalInput")
    # w_pack = [qkv_wT | out_wT] : [256, 1024]; vec_pack cols =
    # qb0..qb5, nw0, nw1, nb0, nb1, ob0, ob1 : [128, 12]
    w_pack = nc.dram_tensor("w_pack", [C, 4 * C], f32, kind="ExternalInput")
    vec_pack = nc.dram_tensor("vec_pack", [128, 12], f32, kind="ExternalInput")
    y = nc.dram_tensor("y", [C, NQ], f32, kind="ExternalOutput")

    with ExitStack() as ctx:
        tc = ctx.enter_context(tile.TileContext(nc))
        const = ctx.enter_context(tc.tile_pool(name="const", bufs=1))
        xqp = ctx.enter_context(tc.tile_pool(name="xqp", bufs=16))
        hp = ctx.enter_context(tc.tile_pool(name="hp", bufs=16))
        kp = ctx.enter_context(tc.tile_pool(name="kp", bufs=2))
        qp = ctx.enter_context(tc.tile_pool(name="qp", bufs=2))
        vtp = ctx.enter_context(tc.tile_pool(name="vtp", bufs=32))
        esp = ctx.enter_context(tc.tile_pool(name="esp", bufs=16))
        smal = ctx.enter_context(tc.tile_pool(name="smal", bufs=2))
        sb2 = ctx.enter_context(tc.tile_pool(name="sb2", bufs=2))
        psp = ctx.enter_context(tc.tile_pool(name="psp", bufs=3, space="PSUM"))
        pvp = ctx.enter_context(tc.tile_pool(name="pvp", bufs=2, space="PSUM"))
        denp = ctx.enter_context(tc.tile_pool(name="denp", bufs=1, space="PSUM"))
        finp = ctx.enter_context(tc.tile_pool(name="finp", bufs=2, space="PSUM"))

        # ---- load x FIRST in [128,512] units round-robin over the three
        #      DMA-capable engines (one queue each); finer units mean the
        #      per-unit bn_stats pipelines tightly with arrivals ----
        xq = [[None] * 8 for _ in range(2)]
        dma_engs = [nc.sync, nc.gpsimd, nc.scalar]
        for u in range(8):
            for ct in range(2):
                t = xqp.tile([128, 512], f16, tag="xq", name=f"xq{ct}_{u}")
                eng = dma_engs[(u * 2 + ct) % 3]
                eng.dma_start(
                    out=t,
                    in_=x16[ct * 128:(ct + 1) * 128, u * 512:(u + 1) * 512])
                xq[ct][u] = t

        # ---- constants (after the x DMA issues: the tiny ind_bwd
        #      SBUF-to-SBUF DMAs would otherwise delay x on the sync queue) ----
        ones_col = const.tile([128, 128], f16, name="ones_col")
        nc.vector.memset(ones_col, 1.0)
        ones_row = const.tile([1, 128], f32, name="ones_row")
        nc.vector.memset(ones_row, 1.0)
        ind_fwd = const.tile([128, 4], f32, name="ind_fwd")   # [ch, grp] one-hot
        nc.vector.memset(ind_fwd, 0.0)
        for g in range(4):
            nc.vector.memset(ind_fwd[g * GS:(g + 1) * GS, g:g + 1], 1.0)
        ind_bwd = const.tile([4, 128], f32, name="ind_bwd")   # [grp, ch] one-hot
        nc.vector.memset(ind_bwd[0:4, :], 0.0)
        for g in range(4):
            # engines can't start at partition g∉{0,32,64,96}; DMA can
            nc.sync.dma_start(out=ind_bwd[g:g + 1, g * GS:(g + 1) * GS],
                              in_=ones_row[0:1, 0:GS])
        eps_t = const.tile([128, 1], f32, name="eps_t")
        nc.vector.memset(eps_t, EPS)

        # ---- packed weights / biases (2 + 1 DMA issues) ----
        wpk = []
        for ci in range(2):
            w = const.tile([128, 4 * C], f32, name=f"wpk{ci}")
            for hh in range(2):
                dma_engs[(ci * 2 + hh) % 3].dma_start(
                    out=w[:, hh * 512:(hh + 1) * 512],
                    in_=w_pack[ci * 128:(ci + 1) * 128,
                               hh * 512:(hh + 1) * 512])
            wpk.append(w)
        vp = const.tile([128, 12], f32, name="vp")
        nc.gpsimd.dma_start(out=vp, in_=vec_pack[:, :])
        wt = [wpk[ci][:, 0:3 * C] for ci in range(2)]
        owt = [wpk[ci][:, 3 * C:4 * C] for ci in range(2)]
        qb = [vp[:, j:j + 1] for j in range(6)]
        nw = [vp[:, 6 + ci:7 + ci] for ci in range(2)]
        nb = [vp[:, 8 + ci:9 + ci] for ci in range(2)]
        ob = [vp[:, 10 + ci:11 + ci] for ci in range(2)]

        # fp16 casts of matmul weights (fp32 on PE = 2 HW passes at half
        # rate); q+k columns cast as soon as their w_pack half lands so the
        # first convs don't wait for the v/out_w half
        wt16, owt16 = [], []
        for ci in range(2):
            w16 = const.tile([128, 3 * C], f16, name=f"wt16_{ci}")
            nc.vector.tensor_copy(out=w16[:, 0:512], in_=wt[ci][:, 0:512])
            nc.vector.tensor_copy(out=w16[:, 512:768], in_=wt[ci][:, 512:768])
            wt16.append(w16)
            o16 = const.tile([128, C], f16, name=f"owt16_{ci}")
            nc.vector.tensor_copy(out=o16, in_=owt[ci])
            owt16.append(o16)
        qb16v = []
        for j in (4, 5):
            t16 = const.tile([128, 1], f16, name=f"qb16_{j}")
            nc.vector.tensor_copy(out=t16, in_=qb[j])
            qb16v.append(t16)

        # one bn_stats per arriving [128,512] unit (DVE)
        stats = [smal.tile([128, 8, 6], f32, tag=f"stats{ct}", name=f"stats{ct}")
                 for ct in range(2)]
        for u in range(8):
            for ct in range(2):
                nc.vector.bn_stats(out=stats[ct][:, u, :], in_=xq[ct][u])

        # paired layout: mv = [m0, m1, v0, v1] via strided aggr outputs, so
        # every chain op handles both ct tiles at once
        mv = smal.tile([128, 4], f32, tag="mv", name="mv0")
        nc.vector.bn_aggr(out=mv[:, 0:4:2], in_=stats[0])   # cols 0,2
        nc.vector.bn_aggr(out=mv[:, 1:4:2], in_=stats[1])   # cols 1,3
        st2 = smal.tile([128, 4], f32, tag="st2", name="st2")  # [m0,m1,E0,E1]
        nc.vector.tensor_copy(out=st2[:, 0:2], in_=mv[:, 0:2])
        nc.vector.tensor_mul(out=st2[:, 2:4], in0=mv[:, 0:2], in1=mv[:, 0:2])
        nc.vector.tensor_add(out=st2[:, 2:4], in0=st2[:, 2:4], in1=mv[:, 2:4])

        # cross-partition group reduce + broadcast back, both cts at once
        gp = psp.tile([4, 4], f32, tag="ps", name="gp")
        nc.tensor.matmul(gp, ind_fwd, st2, start=True, stop=True)
        gsb = smal.tile([4, 4], f32, tag="gsb", name="gsb")
        nc.scalar.mul(out=gsb, in_=gp, mul=1.0 / GS)
        chp = psp.tile([128, 4], f32, tag="ps", name="chp")
        nc.tensor.matmul(chp, ind_bwd, gsb, start=True, stop=True)
        ch = smal.tile([128, 4], f32, tag="ch", name="ch")   # [mu0,mu1,E20,E21]
        nc.vector.tensor_copy(out=ch, in_=chp)
        var = smal.tile([128, 2], f32, tag="var", name="var")
        nc.vector.tensor_mul(out=var, in0=ch[:, 0:2], in1=ch[:, 0:2])
        nc.vector.tensor_sub(out=var, in0=ch[:, 2:4], in1=var)
        std = smal.tile([128, 2], f32, tag="std", name="std")
        nc.scalar.activation(out=std, in_=var, func=Act.Sqrt, bias=eps_t)
        rstd = smal.tile([128, 2], f32, tag="rstd", name="rstd")
        nc.vector.reciprocal(out=rstd, in_=std)
        Aab = smal.tile([128, 2], f32, tag="Aab", name="Aab")
        nc.vector.tensor_mul(out=Aab, in0=vp[:, 6:8], in1=rstd)
        Bab = smal.tile([128, 2], f32, tag="Bab", name="Bab")
        nc.vector.tensor_mul(out=Bab, in0=ch[:, 0:2], in1=Aab)
        nc.vector.tensor_sub(out=Bab, in0=vp[:, 8:10], in1=Bab)
        AB = [(Aab[:, 0:1], Bab[:, 0:1]), (Aab[:, 1:2], Bab[:, 1:2])]

        # apply affine -> fp16 h tiles per 512 cols (ct0 on DVE, ct1 on
        # ACT, in parallel; fine granularity lets convs start per slice)
        h16 = [[None] * 8 for _ in range(2)]
        for s in range(8):
            for ct in range(2):
                ht = hp.tile([128, 512], f16, tag="h16", name=f"h16_{ct}_{s}")
                xsl = xq[ct][s]
                if ct == 0:
                    nc.vector.tensor_scalar(
                        out=ht, in0=xsl,
                        scalar1=AB[ct][0], scalar2=AB[ct][1],
                        op0=Alu.mult, op1=Alu.add)
                else:
                    nc.scalar.activation(
                        out=ht, in_=xsl, func=Act.Identity,
                        bias=AB[ct][1], scale=AB[ct][0])
                h16[ct][s] = ht

        def hsl(ct, nch):  # [128,512] tile for 512-col chunk nch
            return h16[ct][nch]

        def hmt(ct, mt):   # [128,128] slice of h16 for 128-col tile mt
            return h16[ct][mt // 4][:, (mt % 4) * 128:((mt % 4) + 1) * 128]

        # ---- q conv (queries = first NQ cols) ----
        q_sb = [qp.tile([128, NQ], f16, tag="q", name=f"q{co}") for co in range(2)]
        for co in range(2):
            for nch in range(NQ // 512):
                pq = psp.tile([128, 512], f32, tag="ps", name=f"pq{co}_{nch}")
                nc.tensor.matmul(pq, wt16[0][:, co * 128:(co + 1) * 128],
                                 hsl(0, nch), start=True, stop=False)
                nc.tensor.matmul(pq, wt16[1][:, co * 128:(co + 1) * 128],
                                 hsl(1, nch), start=False, stop=True)
                nc.vector.tensor_scalar_add(
                    out=q_sb[co][:, nch * 512:(nch + 1) * 512],
                    in0=pq, scalar1=qb[co])

        # ---- k conv ----
        k_sb = [kp.tile([128, N], f16, tag="k", name=f"k{co}") for co in range(2)]
        for co in range(2):
            for nch in range(N // 512):
                pk = psp.tile([128, 512], f32, tag="ps", name=f"pk{co}_{nch}")
                nc.tensor.matmul(pk, wt16[0][:, 256 + co * 128:256 + (co + 1) * 128],
                                 hsl(0, nch), start=True, stop=False)
                nc.tensor.matmul(pk, wt16[1][:, 256 + co * 128:256 + (co + 1) * 128],
                                 hsl(1, nch), start=False, stop=True)
                nc.vector.tensor_scalar_add(
                    out=k_sb[co][:, nch * 512:(nch + 1) * 512],
                    in0=pk, scalar1=qb[2 + co])

        # ---- vT conv (no bias; folded into effb) ----
        vt_sb = []
        for mt in range(32):
            pvt = psp.tile([128, 256], f32, tag="ps", name=f"pvt{mt}")
            nc.tensor.matmul(pvt, hmt(0, mt), wt16[0][:, 512:768],
                             start=True, stop=False)
            nc.tensor.matmul(pvt, hmt(1, mt), wt16[1][:, 512:768],
                             start=False, stop=True)
            vt = vtp.tile([128, 256], f16, tag="vt", name=f"vt{mt}")
            nc.vector.tensor_copy(out=vt, in_=pvt)
            vt_sb.append(vt)

        # ---- effective output bias: out_w @ v_bias + out_b ----
        effb = []
        for co in range(2):
            pe_ = psp.tile([128, 1], f32, tag="ps", name=f"peffb{co}")
            nc.tensor.matmul(pe_, owt16[0][:, co * 128:(co + 1) * 128], qb16v[0],
                             start=True, stop=False)
            nc.tensor.matmul(pe_, owt16[1][:, co * 128:(co + 1) * 128], qb16v[1],
                             start=False, stop=True)
            e = smal.tile([128, 1], f32, tag="effb", name=f"effb{co}")
            nc.vector.tensor_add(out=e, in0=pe_, in1=ob[co])
            effb.append(e)

        # ---- attention ----
        def st_chunk(t, mch):
            ps_s = psp.tile([128, 512], f32, tag="ps", name=f"s_{t}_{mch}")
            nc.tensor.matmul(ps_s, k_sb[0][:, mch * 128:(mch + 1) * 128],
                             q_sb[0][:, t * 512:(t + 1) * 512],
                             start=True, stop=False)
            nc.tensor.matmul(ps_s, k_sb[1][:, mch * 128:(mch + 1) * 128],
                             q_sb[1][:, t * 512:(t + 1) * 512],
                             start=False, stop=True)
            es = esp.tile([128, 512], f16, tag="es", name=f"es_{t}_{mch}")
            nc.scalar.activation(out=es, in_=ps_s, func=Act.Exp, scale=SCALE)
            return es

        def epilogue(t, pv, den):
            # den arrives already broadcast across partitions ([128,512]);
            # 1/den is entirely off the PE critical path (normalization
            # commutes through the output projection as a per-query scale).
            # The reciprocal and the y chain run in column halves so the
            # final tile's tail pipelines instead of serializing.
            rdb = sb2.tile([128, 512], f32, tag="rdb", name=f"rdb_{t}")
            for hh in range(2):
                nc.vector.reciprocal(out=rdb[:, hh * 256:(hh + 1) * 256],
                                     in_=den[:, hh * 256:(hh + 1) * 256])
            pvsb = []
            for co in range(2):
                p = sb2.tile([128, 512], f16, tag=f"pvsb{co}",
                             name=f"pvsb_{t}_{co}")
                nc.scalar.copy(out=p, in_=pv[co])  # frees the pv psum bank
                pvsb.append(p)
            fins, rsbs = [], []
            for co in range(2):
                fin = finp.tile([128, 512], f32, tag="fin", name=f"fin_{t}_{co}")
                nc.tensor.matmul(fin, owt16[0][:, co * 128:(co + 1) * 128],
                                 pvsb[0], start=True, stop=False)
                nc.tensor.matmul(fin, owt16[1][:, co * 128:(co + 1) * 128],
                                 pvsb[1], start=False, stop=True)
                rsb = sb2.tile([128, 512], f32, tag=f"rsb{co}", name=f"r_{t}_{co}")
                nc.sync.dma_start(
                    out=rsb,
                    in_=x_kv[co * 128:(co + 1) * 128, t * 512:(t + 1) * 512])
                fins.append(fin)
                rsbs.append(rsb)
            for co in range(2):
                t1 = sb2.tile([128, 512], f32, tag=f"t1{co}", name=f"t1_{t}_{co}")
                ysb = sb2.tile([128, 512], f32, tag=f"ysb{co}", name=f"y_{t}_{co}")
                for hh in range(2):
                    sl = slice(hh * 256, (hh + 1) * 256)
                    nc.vector.tensor_mul(out=t1[:, sl], in0=fins[co][:, sl],
                                         in1=rdb[:, sl])
                    nc.vector.scalar_tensor_tensor(
                        out=ysb[:, sl], in0=t1[:, sl], scalar=effb[co],
                        in1=rsbs[co][:, sl], op0=Alu.add, op1=Alu.add)
                    nc.sync.dma_start(
                        out=y[co * 128:(co + 1) * 128,
                              t * 512 + hh * 256:t * 512 + (hh + 1) * 256],
                        in_=ysb[:, sl])

        NT = NQ // 512
        pend = None
        for t in range(NT):
            # first two S^T chunks (and their exps) are emitted BEFORE the
            # previous tile's epilogue so the ACT stream does exp(t,0/1)
            # first and the PE never waits on the pvsb drains
            es_q = {m: st_chunk(t, m) for m in range(4)}
            if pend is not None:
                epilogue(*pend)   # frees the pv psum banks via pvsb copies
            pv = [pvp.tile([128, 512], f32, tag="pv", name=f"pv_{t}_{co}")
                  for co in range(2)]
            den = denp.tile([128, 512], f32, tag="den", name=f"den_{t}")
            grp, e2s = [], []
            pend_den, den_started = None, False

            def emit_den(rhs, stop=False):
                nonlocal den_started
                nc.tensor.matmul(den, ones_col, rhs,
                                 start=not den_started, stop=stop)
                den_started = True

            for mch in range(32):
                es = es_q.pop(mch)
                st, sp = (mch == 0), (mch == 31)
                nc.tensor.matmul(pv[0], vt_sb[mch][:, 0:128], es, start=st, stop=sp)
                nc.tensor.matmul(pv[1], vt_sb[mch][:, 128:256], es, start=st, stop=sp)
                if mch == 28 and pend_den is not None:
                    emit_den(pend_den)
                    pend_den = None
                if mch >= 28:
                    # tail chunks feed den directly: no GpSimd-add latency at
                    # the tile boundary (den gates the epilogue + next tile)
                    emit_den(es, stop=(mch == 31))
                else:
                    # es quad-sums on the otherwise idle GpSimd engine
                    # quarter the den matmul streams through the PE; each
                    # quad's den matmul is deferred one quad so the PE never
                    # waits on the 3-add GpSimd chain
                    grp.append(es)
                    if len(grp) == 2:
                        e2 = esp.tile([128, 512], f16, tag="es2",
                                      name=f"es2_{t}_{mch}")
                        nc.gpsimd.tensor_add(out=e2, in0=grp[0], in1=grp[1])
                        grp, e2s = [], e2s + [e2]
                        if len(e2s) == 2:
                            q4 = mch // 4
                            e4 = esp.tile([128, 512], f16, tag="es4",
                                          name=f"es4_{t}_{q4}")
                            nc.gpsimd.tensor_add(out=e4, in0=e2s[0], in1=e2s[1])
                            e2s = []
                            if pend_den is not None:
                                emit_den(pend_den)
                            pend_den = e4
                if mch + 4 < 32:
                    es_q[mch + 4] = st_chunk(t, mch + 4)
            pend = (t, pv, den)
        epilogue(*pend)

    _split_multiwait_instructions(nc)
    return nc


def _get_nc():
    if "nc" not in _CACHE:
        _CACHE["nc"] = _build()
    return _CACHE["nc"]


def _prep_in_maps(x, norm_w, norm_b, qkv_w, qkv_b, out_w, out_b):
    xr = np.ascontiguousarray(np.asarray(x, dtype=np.float32).reshape(B, C, N))
    wT = np.asarray(qkv_w, np.float32).T
    owT = np.asarray(out_w, np.float32).T
    w_pack = np.ascontiguousarray(np.concatenate([wT, owT], axis=1))
    qb1 = np.asarray(qkv_b, np.float32)
    vec_pack = np.ascontiguousarray(np.stack(
        [qb1[0:128], qb1[128:256], qb1[256:384], qb1[384:512],
         qb1[512:640], qb1[640:768],
         np.asarray(norm_w, np.float32)[0:128],
         np.asarray(norm_w, np.float32)[128:256],
         np.asarray(norm_b, np.float32)[0:128],
         np.asarray(norm_b, np.float32)[128:256],
         np.asarray(out_b, np.float32)[0:128],
         np.asarray(out_b, np.float32)[128:256]], axis=1))

    in_maps = []
    for c in range(8):
        b, h = divmod(c, 2)
        off = h * NQ
        xb = xr[b]
        if off:
            xroll = np.ascontiguousarray(
                np.concatenate([xb[:, off:], xb[:, :off]], axis=1))
        else:
            xroll = xb
        in_maps.append({"x_kv": xroll, "x16": xroll.astype(np.float16),
                        "w_pack": w_pack, "vec_pack": vec_pack})
    return in_maps


def kernel(x, norm_w, norm_b, qkv_w, qkv_b, out_w, out_b):
    from concourse.bass_utils import run_bass_kernel_spmd

    nc = _get_nc()
    in_maps = _prep_in_maps(x, norm_w, norm_b, qkv_w, qkv_b, out_w, out_b)
    res = run_bass_kernel_spmd(nc, in_maps, core_ids=list(range(8)))
    out = np.empty((B, C, N), np.float32)
    for c in range(8):
        b, h = divmod(c, 2)
        off = h * NQ
        out[b][:, off:off + NQ] = res.results[c]["y"]
    return out.reshape(B, C, 16, 16, 16)


# revision 36
# speedup vs baseline: 1.2239x; 1.2075x over previous
"""AttentionBlock3D (GroupNorm + single-head attention over 4096 tokens + residual)
on 8 Trainium2 NeuronCores.

Sharding: core c handles sample b=c//2 and query half h=c%2 (2048 queries).
The host rotates each sample's token axis so that each core's queries are
columns 0..2048 of its x_kv input (attention is permutation-invariant over
keys, and groupnorm stats are permutation-invariant), letting a single SPMD
program serve all 8 cores with no dynamic offsets.

Device-side math per core:
  h = groupnorm(x)                   fp32 stats (bn_stats + indicator-matmul
                                     cross-partition group reduce), fp16 output
  q = Wq h + bq  [256,2048] fp16     (host passes qkv_w^T; no on-chip transposes)
  k = Wk h + bk  [256,4096] fp16
  vT = (Wv h)^T  [4096,256] fp16     (computed directly in transposed layout)
  S^T tiles [128 keys, 512 queries] -> es = exp(S^T * C^-0.5) fp16
                                     (no max subtraction: |scores*scale| <~ 8)
  pv[c,q]  = sum_m vT[m,c] es[m,q]   (v chunks stationary, es streams, fp32 psum)
  den[q]   = sum_m es[m,q]           (ones-column matmul)
  fin      = out_w @ pv              (normalization deferred: a per-query column
                                      scale commutes through the projection)
  y = fin * (1/den bcast) + (out_w @ bv + ob) + x   (v-bias folded via sum(A)=1)

All matmuls run in fp16 (fp32 on the PE lowers to 2 HW passes at half rate);
PSUM accumulation is fp32 throughout, softmax statistics are fp32.
"""
import numpy as np

_CACHE: dict = {}

B, C, N = 4, 256, 4096
NQ = 2048          # queries per core
G = 8              # groups
GS = 32            # channels per group
EPS = 1e-5
SCALE = C ** -0.5


def _install_tile_drain_patch():
    """This container's walrus only supports ONE sync-wait command per
    instruction; TileContext's final drain carries one wait per live proc and
    fails codegen. Re-emit the waits as standalone sem-wait instructions."""
    import concourse.tile as tile_mod
    from concourse.vector_clock import ScopedClock

    if getattr(tile_mod.TileContext, "_ant_drain_patched", False):
        return

    def _patched(self, tick_clock, wait_clock):
        nc = self.nc
        probe = nc.sync.nop(nofuse=True)
        wait_clock.add_sem_waits(
            probe.ins, ScopedClock({None: tick_clock.global_clock})
        )
        waits = list(probe.ins.sync_info.on_wait or [])
        if len(waits) > 1:
            probe.ins.sync_info.on_wait = []
            num2h = {}
            assert self.sems is not None
            for h in self.sems.allocated().values():
                num2h[h.num] = h
            for h in self.sems.swdge_block():
                num2h.setdefault(h.num, h)
            for w in waits:
                h = num2h.get(w.id)
                assert h is not None, f"sem id {w.id} ({w.ant_name}) has no handle"
                nc.sync.wait_op(h, w.wait_value, "sem-ge")
        nc.sync.drain()
        nc.all_engine_barrier()
        assert self.sems is not None
        popped = nc._tile_sem_poison_stack.pop()
        assert popped is self._sem_poison
        nc.clear_and_free_semaphores(list(self.sems.allocated().values()))

    tile_mod.TileContext._drain_and_barrier = _patched
    tile_mod.TileContext._ant_drain_patched = True


def _split_multiwait_instructions(nc):
    """This walrus build supports only one sync-wait command per instruction.
    Hoist extra waits into standalone InstEventSemaphore waits on the same
    engine immediately before the instruction (sequential single waits are
    equivalent to one multi-wait)."""
    import concourse.mybir as mybir

    n = 0
    for f in nc.m.functions:
        for bb in f.blocks:
            new = []
            for ins in bb.instructions:
                si = ins.sync_info
                waits = list(si.on_wait) if si else []
                if len(waits) > 1:
                    for w in waits[:-1]:
                        nop = mybir.InstEventSemaphore(
                            name=f"WSPLIT-{n}", ins=[], outs=[])
                        n += 1
                        nop.engine = ins.engine
                        nop.sync_info = mybir.SyncInfo(on_wait=[w], on_update=[])
                        new.append(nop)
                    si.on_wait = [waits[-1]]
                new.append(ins)
            bb.instructions[:] = new


def _build():
    import concourse.bass as bass
    import concourse.tile as tile
    import concourse.mybir as mybir
    from contextlib import ExitStack

    _install_tile_drain_patch()

    f32 = mybir.dt.float32
    f16 = mybir.dt.float16
    Act = mybir.ActivationFunctionType
    Alu = mybir.AluOpType

    nc = bass.Bass()
    x_kv = nc.dram_tensor("x_kv", [C, N], f32, kind="ExternalInput")
    x16 = nc.dram_tensor("x16", [C, N], f16, kind="ExternalInput")
    # w_pack = [qkv_wT | out_wT] : [256, 1024]; vec_pack cols =
    # qb0..qb5, nw0, nw1, nb0, nb1, ob0, ob1 : [128, 12]
    w_pack = nc.dram_tensor("w_pack", [C, 4 * C], f32, kind="ExternalInput")
    vec_pack = nc.dram_tensor("vec_pack", [128, 12], f32, kind="ExternalInput")
    y = nc.dram_tensor("y", [C, NQ], f32, kind="ExternalOutput")

    with ExitStack() as ctx:
        tc = ctx.enter_context(tile.TileContext(nc))
        const = ctx.enter_context(tc.tile_pool(name="const", bufs=1))
        xqp = ctx.enter_context(tc.tile_pool(name="xqp", bufs=16))
        hp = ctx.enter_context(tc.tile_pool(name="hp", bufs=16))
        kp = ctx.enter_context(tc.tile_pool(name="kp", bufs=2))
        qp = ctx.enter_context(tc.tile_pool(name="qp", bufs=2))
        vtp = ctx.enter_context(tc.tile_pool(name="vtp", bufs=32))
        esp = ctx.enter_context(tc.tile_pool(name="esp", bufs=16))
        smal = ctx.enter_context(tc.tile_pool(name="smal", bufs=2))
        sb2 = ctx.enter_context(tc.tile_pool(name="sb2", bufs=2))
        psp = ctx.enter_context(tc.tile_pool(name="psp", bufs=3, space="PSUM"))
        pvp = ctx.enter_context(tc.tile_pool(name="pvp", bufs=2, space="PSUM"))
        denp = ctx.enter_context(tc.tile_pool(name="denp", bufs=1, space="PSUM"))
        finp = ctx.enter_context(tc.tile_pool(name="finp", bufs=2, space="PSUM"))

        # ---- load x FIRST in [128,512] units round-robin over the three
        #      DMA-capable engines (one queue each); finer units mean the
        #      per-unit bn_stats pipelines tightly with arrivals ----
        xq = [[None] * 8 for _ in range(2)]
        dma_engs = [nc.sync, nc.gpsimd, nc.scalar]
        for u in range(8):
            for ct in range(2):
                t = xqp.tile([128, 512], f16, tag="xq", name=f"xq{ct}_{u}")
                eng = dma_engs[(u * 2 + ct) % 3]
                eng.dma_start(
                    out=t,
                    in_=x16[ct * 128:(ct + 1) * 128, u * 512:(u + 1) * 512])
                xq[ct][u] = t

        # ---- constants (after the x DMA issues: the tiny ind_bwd
        #      SBUF-to-SBUF DMAs would otherwise delay x on the sync queue) ----
        ones_col = const.tile([128, 128], f16, name="ones_col")
        nc.vector.memset(ones_col, 1.0)
        ones_row = const.tile([1, 128], f32, name="ones_row")
        nc.vector.memset(ones_row, 1.0)
        ind_fwd = const.tile([128, 4], f32, name="ind_fwd")   # [ch, grp] one-hot
        nc.vector.memset(ind_fwd, 0.0)
        for g in range(4):
            nc.vector.memset(ind_fwd[g * GS:(g + 1) * GS, g:g + 1], 1.0)
        ind_bwd = const.tile([4, 128], f32, name="ind_bwd")   # [grp, ch] one-hot
        nc.vector.memset(ind_bwd[0:4, :], 0.0)
        for g in range(4):
            # engines can't start at partition g∉{0,32,64,96}; DMA can
            nc.sync.dma_start(out=ind_bwd[g:g + 1, g * GS:(g + 1) * GS],
                              in_=ones_row[0:1, 0:GS])
        eps_t = const.tile([128, 1], f32, name="eps_t")
        nc.vector.memset(eps_t, EPS)

        # ---- packed weights / biases (2 + 1 DMA issues) ----
        wpk = []
        for ci in range(2):
            w = const.tile([128, 4 * C], f32, name=f"wpk{ci}")
            for hh in range(2):
                dma_engs[(ci * 2 + hh) % 3].dma_start(
                    out=w[:, hh * 512:(hh + 1) * 512],
                    in_=w_pack[ci * 128:(ci + 1) * 128,
                               hh * 512:(hh + 1) * 512])
            wpk.append(w)
        vp = const.tile([128, 12], f32, name="vp")
        nc.gpsimd.dma_start(out=vp, in_=vec_pack[:, :])
        wt = [wpk[ci][:, 0:3 * C] for ci in range(2)]
        owt = [wpk[ci][:, 3 * C:4 * C] for ci in range(2)]
        qb = [vp[:, j:j + 1] for j in range(6)]
        nw = [vp[:, 6 + ci:7 + ci] for ci in range(2)]
        nb = [vp[:, 8 + ci:9 + ci] for ci in range(2)]
        ob = [vp[:, 10 + ci:11 + ci] for ci in range(2)]

        # fp16 casts of matmul weights (fp32 on PE = 2 HW passes at half
        # rate); q+k columns cast as soon as their w_pack half lands so the
        # first convs don't wait for the v/out_w half
        wt16, owt16 = [], []
        for ci in range(2):
            w16 = const.tile([128, 3 * C], f16, name=f"wt16_{ci}")
            nc.gpsimd.tensor_copy(out=w16[:, 0:512], in_=wt[ci][:, 0:512])
            nc.gpsimd.tensor_copy(out=w16[:, 512:768], in_=wt[ci][:, 512:768])
            wt16.append(w16)
            o16 = const.tile([128, C], f16, name=f"owt16_{ci}")
            nc.gpsimd.tensor_copy(out=o16, in_=owt[ci])
            owt16.append(o16)
        qb16v = []
        for j in (4, 5):
            t16 = const.tile([128, 1], f16, name=f"qb16_{j}")
            nc.vector.tensor_copy(out=t16, in_=qb[j])
            qb16v.append(t16)

        # one bn_stats per arriving [128,512] unit (DVE)
        stats = [smal.tile([128, 8, 6], f32, tag=f"stats{ct}", name=f"stats{ct}")
                 for ct in range(2)]
        for u in range(8):
            for ct in range(2):
                nc.vector.bn_stats(out=stats[ct][:, u, :], in_=xq[ct][u])

        # paired layout: mv = [m0, m1, v0, v1] via strided aggr outputs, so
        # every chain op handles both ct tiles at once
        mv = smal.tile([128, 4], f32, tag="mv", name="mv0")
        nc.vector.bn_aggr(out=mv[:, 0:4:2], in_=stats[0])   # cols 0,2
        nc.vector.bn_aggr(out=mv[:, 1:4:2], in_=stats[1])   # cols 1,3
        st2 = smal.tile([128, 4], f32, tag="st2", name="st2")  # [m0,m1,E0,E1]
        nc.vector.tensor_copy(out=st2[:, 0:2], in_=mv[:, 0:2])
        nc.vector.tensor_mul(out=st2[:, 2:4], in0=mv[:, 0:2], in1=mv[:, 0:2])
        nc.vector.tensor_add(out=st2[:, 2:4], in0=st2[:, 2:4], in1=mv[:, 2:4])

        # cross-partition group reduce + broadcast back, both cts at once
        gp = psp.tile([4, 4], f32, tag="ps", name="gp")
        nc.tensor.matmul(gp, ind_fwd, st2, start=True, stop=True)
        gsb = smal.tile([4, 4], f32, tag="gsb", name="gsb")
        nc.scalar.mul(out=gsb, in_=gp, mul=1.0 / GS)
        chp = psp.tile([128, 4], f32, tag="ps", name="chp")
        nc.tensor.matmul(chp, ind_bwd, gsb, start=True, stop=True)
        ch = smal.tile([128, 4], f32, tag="ch", name="ch")   # [mu0,mu1,E20,E21]
        nc.vector.tensor_copy(out=ch, in_=chp)
        var = smal.tile([128, 2], f32, tag="var", name="var")
        nc.vector.tensor_mul(out=var, in0=ch[:, 0:2], in1=ch[:, 0:2])
        nc.vector.tensor_sub(out=var, in0=ch[:, 2:4], in1=var)
        std = smal.tile([128, 2], f32, tag="std", name="std")
        nc.scalar.activation(out=std, in_=var, func=Act.Sqrt, bias=eps_t)
        rstd = smal.tile([128, 2], f32, tag="rstd", name="rstd")
        nc.vector.reciprocal(out=rstd, in_=std)
        Aab = smal.tile([128, 2], f32, tag="Aab", name="Aab")
        nc.vector.tensor_mul(out=Aab, in0=vp[:, 6:8], in1=rstd)
        Bab = smal.tile([128, 2], f32, tag="Bab", name="Bab")
        nc.vector.tensor_mul(out=Bab, in0=ch[:, 0:2], in1=Aab)
        nc.vector.tensor_sub(out=Bab, in0=vp[:, 8:10], in1=Bab)
        AB = [(Aab[:, 0:1], Bab[:, 0:1]), (Aab[:, 1:2], Bab[:, 1:2])]

        # apply affine -> fp16 h tiles per 512 cols (ct0 on DVE, ct1 on
        # ACT, in parallel; fine granularity lets convs start per slice)
        h16 = [[None] * 8 for _ in range(2)]
        for s in range(8):
            for ct in range(2):
                ht = hp.tile([128, 512], f16, tag="h16", name=f"h16_{ct}_{s}")
                xsl = xq[ct][s]
                r = (s * 2 + ct) % 4
                if r == 0 or r == 2:
                    eng_ts = nc.vector if r == 0 else nc.gpsimd
                    eng_ts.tensor_scalar(
                        out=ht, in0=xsl,
                        scalar1=AB[ct][0], scalar2=AB[ct][1],
                        op0=Alu.mult, op1=Alu.add)
                else:
                    nc.scalar.activation(
                        out=ht, in_=xsl, func=Act.Identity,
                        bias=AB[ct][1], scale=AB[ct][0])
                h16[ct][s] = ht

        def hsl(ct, nch):  # [128,512] tile for 512-col chunk nch
            return h16[ct][nch]

        def hmt(ct, mt):   # [128,128] slice of h16 for 128-col tile mt
            return h16[ct][mt // 4][:, (mt % 4) * 128:((mt % 4) + 1) * 128]

        # ---- q conv (queries = first NQ cols) ----
        q_sb = [qp.tile([128, NQ], f16, tag="q", name=f"q{co}") for co in range(2)]
        for co in range(2):
            for nch in range(NQ // 512):
                pq = psp.tile([128, 512], f32, tag="ps", name=f"pq{co}_{nch}")
                nc.tensor.matmul(pq, wt16[0][:, co * 128:(co + 1) * 128],
                                 hsl(0, nch), start=True, stop=False)
                nc.tensor.matmul(pq, wt16[1][:, co * 128:(co + 1) * 128],
                                 hsl(1, nch), start=False, stop=True)
                nc.vector.tensor_scalar_add(
                    out=q_sb[co][:, nch * 512:(nch + 1) * 512],
                    in0=pq, scalar1=qb[co])

        # ---- k conv ----
        k_sb = [kp.tile([128, N], f16, tag="k", name=f"k{co}") for co in range(2)]
        for co in range(2):
            for nch in range(N // 512):
                pk = psp.tile([128, 512], f32, tag="ps", name=f"pk{co}_{nch}")
                nc.tensor.matmul(pk, wt16[0][:, 256 + co * 128:256 + (co + 1) * 128],
                                 hsl(0, nch), start=True, stop=False)
                nc.tensor.matmul(pk, wt16[1][:, 256 + co * 128:256 + (co + 1) * 128],
                                 hsl(1, nch), start=False, stop=True)
                nc.vector.tensor_scalar_add(
                    out=k_sb[co][:, nch * 512:(nch + 1) * 512],
                    in0=pk, scalar1=qb[2 + co])

        # ---- vT conv (no bias; folded into effb) ----
        vt_sb = []
        for mt in range(32):
            pvt = psp.tile([128, 256], f32, tag="ps", name=f"pvt{mt}")
            nc.tensor.matmul(pvt, hmt(0, mt), wt16[0][:, 512:768],
                             start=True, stop=False)
            nc.tensor.matmul(pvt, hmt(1, mt), wt16[1][:, 512:768],
                             start=False, stop=True)
            vt = vtp.tile([128, 256], f16, tag="vt", name=f"vt{mt}")
            nc.vector.tensor_copy(out=vt, in_=pvt)
            vt_sb.append(vt)

        # ---- effective output bias: out_w @ v_bias + out_b ----
        effb = []
        for co in range(2):
            pe_ = psp.tile([128, 1], f32, tag="ps", name=f"peffb{co}")
            nc.tensor.matmul(pe_, owt16[0][:, co * 128:(co + 1) * 128], qb16v[0],
                             start=True, stop=False)
            nc.tensor.matmul(pe_, owt16[1][:, co * 128:(co + 1) * 128], qb16v[1],
                             start=False, stop=True)
            e = smal.tile([128, 1], f32, tag="effb", name=f"effb{co}")
            nc.vector.tensor_add(out=e, in0=pe_, in1=ob[co])
            effb.append(e)

        # ---- attention ----
        def st_chunk(t, mch):
            ps_s = psp.tile([128, 512], f32, tag="ps", name=f"s_{t}_{mch}")
            nc.tensor.matmul(ps_s, k_sb[0][:, mch * 128:(mch + 1) * 128],
                             q_sb[0][:, t * 512:(t + 1) * 512],
                             start=True, stop=False)
            nc.tensor.matmul(ps_s, k_sb[1][:, mch * 128:(mch + 1) * 128],
                             q_sb[1][:, t * 512:(t + 1) * 512],
                             start=False, stop=True)
            es = esp.tile([128, 512], f16, tag="es", name=f"es_{t}_{mch}")
            nc.scalar.activation(out=es, in_=ps_s, func=Act.Exp, scale=SCALE)
            return es

        def epilogue(t, pv, den):
            # den arrives already broadcast across partitions ([128,512]);
            # 1/den is entirely off the PE critical path (normalization
            # commutes through the output projection as a per-query scale).
            # The reciprocal and the y chain run in column halves so the
            # final tile's tail pipelines instead of serializing.
            rdb = sb2.tile([128, 512], f32, tag="rdb", name=f"rdb_{t}")
            for hh in range(2):
                nc.vector.reciprocal(out=rdb[:, hh * 256:(hh + 1) * 256],
                                     in_=den[:, hh * 256:(hh + 1) * 256])
            pvsb = []
            for co in range(2):
                p = sb2.tile([128, 512], f16, tag=f"pvsb{co}",
                             name=f"pvsb_{t}_{co}")
                nc.scalar.copy(out=p, in_=pv[co])  # frees the pv psum bank
                pvsb.append(p)
            fins, rsbs = [], []
            for co in range(2):
                fin = finp.tile([128, 512], f32, tag="fin", name=f"fin_{t}_{co}")
                nc.tensor.matmul(fin, owt16[0][:, co * 128:(co + 1) * 128],
                                 pvsb[0], start=True, stop=False)
                nc.tensor.matmul(fin, owt16[1][:, co * 128:(co + 1) * 128],
                                 pvsb[1], start=False, stop=True)
                rsb = sb2.tile([128, 512], f32, tag=f"rsb{co}", name=f"r_{t}_{co}")
                nc.sync.dma_start(
                    out=rsb,
                    in_=x_kv[co * 128:(co + 1) * 128, t * 512:(t + 1) * 512])
                fins.append(fin)
                rsbs.append(rsb)
            for co in range(2):
                t1 = sb2.tile([128, 512], f32, tag=f"t1{co}", name=f"t1_{t}_{co}")
                ysb = sb2.tile([128, 512], f32, tag=f"ysb{co}", name=f"y_{t}_{co}")
                for hh in range(2):
                    sl = slice(hh * 256, (hh + 1) * 256)
                    nc.vector.tensor_mul(out=t1[:, sl], in0=fins[co][:, sl],
                                         in1=rdb[:, sl])
                    nc.vector.scalar_tensor_tensor(
                        out=ysb[:, sl], in0=t1[:, sl], scalar=effb[co],
                        in1=rsbs[co][:, sl], op0=Alu.add, op1=Alu.add)
                    nc.sync.dma_start(
                        out=y[co * 128:(co + 1) * 128,
                              t * 512 + hh * 256:t * 512 + (hh + 1) * 256],
                        in_=ysb[:, sl])

        NT = NQ // 512
        pend = None
        for t in range(NT):
            # first two S^T chunks (and their exps) are emitted BEFORE the
            # previous tile's epilogue so the ACT stream does exp(t,0/1)
            # first and the PE never waits on the pvsb drains
            es_q = {m: st_chunk(t, m) for m in range(4)}
            if pend is not None:
                epilogue(*pend)   # frees the pv psum banks via pvsb copies
            pv = [pvp.tile([128, 512], f32, tag="pv", name=f"pv_{t}_{co}")
                  for co in range(2)]
            den = denp.tile([128, 512], f32, tag="den", name=f"den_{t}")
            grp, e2s = [], []
            pend_den, den_started = None, False

            def emit_den(rhs, stop=False):
                nonlocal den_started
                nc.tensor.matmul(den, ones_col, rhs,
                                 start=not den_started, stop=stop)
                den_started = True

            for mch in range(32):
                es = es_q.pop(mch)
                st, sp = (mch == 0), (mch == 31)
                nc.tensor.matmul(pv[0], vt_sb[mch][:, 0:128], es, start=st, stop=sp)
                nc.tensor.matmul(pv[1], vt_sb[mch][:, 128:256], es, start=st, stop=sp)
                if mch == 28 and pend_den is not None:
                    emit_den(pend_den)
                    pend_den = None
                if mch >= 28:
                    # tail chunks feed den directly: no GpSimd-add latency at
                    # the tile boundary (den gates the epilogue + next tile)
                    emit_den(es, stop=(mch == 31))
                else:
                    # es quad-sums on the otherwise idle GpSimd engine
                    # quarter the den matmul streams through the PE; each
                    # quad's den matmul is deferred one quad so the PE never
                    # waits on the 3-add GpSimd chain
                    grp.append(es)
                    if len(grp) == 2:
                        e2 = esp.tile([128, 512], f16, tag="es2",
                                      name=f"es2_{t}_{mch}")
                        nc.gpsimd.tensor_add(out=e2, in0=grp[0], in1=grp[1])
                        grp, e2s = [], e2s + [e2]
                        if len(e2s) == 2:
                            q4 = mch // 4
                            e4 = esp.tile([128, 512], f16, tag="es4",
                                          name=f"es4_{t}_{q4}")
                            nc.gpsimd.tensor_add(out=e4, in0=e2s[0], in1=e2s[1])
                            e2s = []
                            if pend_den is not None:
                                emit_den(pend_den)
                            pend_den = e4
                if mch + 4 < 32:
                    es_q[mch + 4] = st_chunk(t, mch + 4)
            pend = (t, pv, den)
        epilogue(*pend)

    _split_multiwait_instructions(nc)
    return nc


def _get_nc():
    if "nc" not in _CACHE:
        _CACHE["nc"] = _build()
    return _CACHE["nc"]


def _prep_in_maps(x, norm_w, norm_b, qkv_w, qkv_b, out_w, out_b):
    xr = np.ascontiguousarray(np.asarray(x, dtype=np.float32).reshape(B, C, N))
    wT = np.asarray(qkv_w, np.float32).T
    owT = np.asarray(out_w, np.float32).T
    w_pack = np.ascontiguousarray(np.concatenate([wT, owT], axis=1))
    qb1 = np.asarray(qkv_b, np.float32)
    vec_pack = np.ascontiguousarray(np.stack(
        [qb1[0:128], qb1[128:256], qb1[256:384], qb1[384:512],
         qb1[512:640], qb1[640:768],
         np.asarray(norm_w, np.float32)[0:128],
         np.asarray(norm_w, np.float32)[128:256],
         np.asarray(norm_b, np.float32)[0:128],
         np.asarray(norm_b, np.float32)[128:256],
         np.asarray(out_b, np.float32)[0:128],
         np.asarray(out_b, np.float32)[128:256]], axis=1))

    in_maps = []
    for c in range(8):
        b, h = divmod(c, 2)
        off = h * NQ
        xb = xr[b]
        if off:
            xroll = np.ascontiguousarray(
                np.concatenate([xb[:, off:], xb[:, :off]], axis=1))
        else:
            xroll = xb
        in_maps.append({"x_kv": xroll, "x16": xroll.astype(np.float16),
                        "w_pack": w_pack, "vec_pack": vec_pack})
    return in_maps


def kernel(x, norm_w, norm_b, qkv_w, qkv_b, out_w, out_b):
    from concourse.bass_utils import run_bass_kernel_spmd

    nc = _get_nc()
    in_maps = _prep_in_maps(x, norm_w, norm_b, qkv_w, qkv_b, out_w, out_b)
    res = run_bass_kernel_spmd(nc, in_maps, core_ids=list(range(8)))
    out = np.empty((B, C, N), np.float32)
    for c in range(8):
        b, h = divmod(c, 2)
        off = h * NQ
        out[b][:, off:off + NQ] = res.results[c]["y"]
    return out.reshape(B, C, 16, 16, 16)
